# revision 23
# baseline (speedup 1.0000x reference)
"""Bass/Tile MHA kernel for trn2 — builder + host shard/unshard helpers.

Per-core work (8 cores): core c handles batch b=c//2, head-group g=c%2
(8 of 16 heads). v2 design:

- Q^T/K^T are PAIR-PACKED on partitions: head A dims at rows 0-47
  (pad 48-63 = 0), head B at rows 64-111 (pad 112-127 = 0). The QK^T
  matmuls then contract K=64 per head using 64x64 PE array tiling:
  4 concurrent matmuls per k-tile (2 heads x 2 k-token halves) cover
  128 k-tokens x 512 q in ~216ns — 2x less PE time than padding K to
  128. AV matmuls also run as 4 concurrent 64x64 tiles (2 heads x 2
  token halves); the token halves accumulate into separate PSUM banks
  (av_lo/av_hi) which VectorE sums during the softmax normalize.
  All attention matmuls share one (64,64) tiling mode => no PE drains.
- Projections run as batched (128,128)-mode matmul groups between
  attention sections (mode switches cost ~300ns, so they are batched,
  never interleaved per k-tile).
- exp processes BOTH heads' scores in ONE instruction over a 2-bank
  PSUM tile [128, 2, 512] (halves instruction overhead); split between
  ScalarE true-exp and VectorE Schraudolph fast-exp per-kt (tunable).
- No big zero-memsets: the projection activation writes all 128
  partitions; pad rows get bias 0 on zero wqk columns => exact zeros.

Dataflow (all matmuls bf16 in / fp32 PSUM accumulate):
  qkT[d_h, t]  = w_qk^T x + b      (pair-packed, full-128 activation)
  V[t, d_v]    = x w_v             (ones col 0 gives softmax denoms)
  S^T[k, q]    = (K^T)^T Q^T       (4x 64x64 concurrent tiles)
  P^T          = exp(S^T)          (ScalarE exp / VectorE fast-exp)
  av_lo/hi     = (V|1)^T P^T       (4x 64x64 tiles, token halves)
  outT         = (av_lo+av_hi) * bcast(1/l)
  y[t, j]      = outT^T w_out      (+ b_out and cross-core sum on host)
"""

import math

import numpy as np
import ml_dtypes

import concourse.bass as bass
import concourse.mybir as mybir
import concourse.tile as tile
from concourse import bacc

F32 = mybir.dt.float32
BF16 = mybir.dt.bfloat16
I16 = mybir.dt.int16
AF = mybir.ActivationFunctionType
OP = mybir.AluOpType

DIM = 768
PH = 48
NP = 4          # head pairs per core
HC = 8          # heads per core
NDT = DIM // 128  # 6 contraction tiles for the projections

# Schraudolph fast-exp in bf16 bit space: bits = round(x*128/ln2 + (127*128 - C))
SCH_A = 128.0 / math.log(2.0)
SCH_C = 4.7
# +0.5: the fp32->int16 convert truncates, this re-centers it to round-nearest
SCH_B = 127.0 * 128.0 - SCH_C + 0.5

# Which kt slots (mod 16) run fast-exp on VectorE instead of ScalarE exp.
# qg==0 attention overlaps the next pair's projections (ScalarE busier
# with activation copies), so it gives VectorE a larger share.
DVE_SLOTS_QG0 = frozenset({1, 3, 5, 7, 9, 11, 13, 15})
DVE_SLOTS = frozenset({2, 5, 7, 10, 13, 15})


def build_kernel(T=2048, num_devices=8,
                 dve_slots_qg0=DVE_SLOTS_QG0, dve_slots=DVE_SLOTS):
    """Returns compiled Bacc module."""
    KT = T // 128                 # k-tiles (token tiles)
    QCW = min(512, T)             # q chunk width (one PSUM bank)
    NQG = T // QCW                # q groups, one chunk each

    nc = bacc.Bacc("TRN2", target_bir_lowering=False, debug=False,
                   num_devices=num_devices)

    xt_d = nc.dram_tensor("xt", (DIM, T), BF16, kind="ExternalInput")
    wqk_d = nc.dram_tensor("wqk", (DIM, NP * 2 * 128), BF16, kind="ExternalInput")
    wv_d = nc.dram_tensor("wv", (DIM, HC * PH), BF16, kind="ExternalInput")
    wo_d = nc.dram_tensor("wo", (NP * 128, DIM), BF16, kind="ExternalInput")
    bqk_d = nc.dram_tensor("bqk", (128, NP * 2), F32, kind="ExternalInput")
    y_d = nc.dram_tensor("y", (T, DIM), F32, kind="ExternalOutput")

    with tile.TileContext(nc) as tc:
        with (
            tc.tile_pool(name="const", bufs=1) as cpool,
            tc.tile_pool(name="pt", bufs=20) as ptpool,
            tc.tile_pool(name="norm", bufs=2) as npool,
            tc.tile_pool(name="ysb", bufs=2) as ypool,
            tc.tile_pool(name="st", bufs=3, space="PSUM") as stpool,
            tc.tile_pool(name="av", bufs=2, space="PSUM") as avpool,
        ):
            # ---- persistent SBUF tensors ----
            xt_sb = cpool.tile([128, NDT, T], BF16, tag="xt")
            wqk_sb = cpool.tile([128, NDT, NP * 2 * 128], BF16, tag="wqk")
            wv_sb = cpool.tile([128, NDT, HC * PH], BF16, tag="wv")
            wo_sb = cpool.tile([128, NP, DIM], BF16, tag="wo")
            bqk_sb = cpool.tile([128, NP * 2], F32, tag="bqk")
            # pair-packed Q^T/K^T: rows 0-47 head A (48-63 zero), 64-111
            # head B (112-127 zero); slot 0 = Q^T, slot 1 = K^T
            qk_sb = cpool.tile([128, NP, 2, T], BF16, tag="qk")
            # V' columns per head: 0 = ones (softmax denominator lands on
            # PSUM row 0 / 64 of av_lo+av_hi), 1-48 = V, 49-63 = zero
            v_sb = cpool.tile([128, KT, HC, 64], BF16, tag="v")
            outT_sb = cpool.tile([128, NP, T], BF16, tag="outT")

            # ---- input DMAs ----
            # xt + wqk + bqk first (the first projection chunk needs them),
            # wv/wo later (only needed once attention starts)
            nc.sync.dma_start(bqk_sb[:], bqk_d[:])
            # split by partition halves (full DRAM rows stay contiguous) so
            # transfers spread across more queues and land sooner
            for dt_i in range(NDT):
                for h in range(2):
                    ps_ = np.s_[h * 64:(h + 1) * 64]
                    rs = np.s_[dt_i * 128 + h * 64:dt_i * 128 + (h + 1) * 64]
                    nc.sync.dma_start(xt_sb[ps_, dt_i, :], xt_d[rs, :])
                    nc.sync.dma_start(wqk_sb[ps_, dt_i, :], wqk_d[rs, :])
            for dt_i in range(NDT):
                nc.sync.dma_start(wv_sb[:, dt_i, :], wv_d[dt_i * 128:(dt_i + 1) * 128, :])
            for p in range(NP):
                nc.sync.dma_start(wo_sb[:, p, :], wo_d[p * 128:(p + 1) * 128, :])

            # ones column for the softmax-denominator trick; zero pads keep
            # av rows 49-63 / 113-127 finite (they multiply rbc pads below)
            nc.gpsimd.memset(v_sb[:, :, :, 0:1], 1.0)
            nc.gpsimd.memset(v_sb[:, :, :, PH + 1:64], 0.0)

            def qkT_proj_chunks(p):
                """4 emit-closures, one per (q/k slot, 2-chunk group)."""
                chunks = []
                for sl in range(2):
                    for tc2 in range(T // (2 * QCW)):
                        def emit(sl=sl, tc2=tc2):
                            ps = stpool.tile([128, 2, QCW], F32, tag="st",
                                             name="pjqk")
                            col0 = (p * 2 + sl) * 128
                            for half in range(2):
                                tcI = tc2 * 2 + half
                                for dt_i in range(NDT):
                                    nc.tensor.matmul(
                                        ps[:, half, :],
                                        wqk_sb[:, dt_i, col0:col0 + 128],
                                        xt_sb[:, dt_i, tcI * QCW:(tcI + 1) * QCW],
                                        start=(dt_i == 0), stop=(dt_i == NDT - 1),
                                        skip_group_check=True)
                            t0 = tc2 * 2 * QCW
                            nc.scalar.activation(
                                qk_sb[:, p, sl, t0:t0 + 2 * QCW].rearrange(
                                    "p (a b) -> p a b", a=2),
                                ps[:, :, :], AF.Identity,
                                bias=bqk_sb[:, p * 2 + sl:p * 2 + sl + 1])
                        chunks.append(emit)
                return chunks

            def vproj_chunks():
                """8 emit-closures, one per tt pair."""
                chunks = []
                for tp in range(KT // 2):
                    def emit(tp=tp):
                        ps = stpool.tile([128, 2, QCW], F32, tag="st",
                                         name="pjv")
                        for half in range(2):
                            tt = tp * 2 + half
                            for dt_i in range(NDT):
                                nc.tensor.matmul(
                                    ps[:, half, 0:HC * PH],
                                    xt_sb[:, dt_i, tt * 128:(tt + 1) * 128],
                                    wv_sb[:, dt_i, :],
                                    start=(dt_i == 0), stop=(dt_i == NDT - 1),
                                    skip_group_check=True)
                        for half in range(2):
                            tt = tp * 2 + half
                            nc.scalar.activation(
                                v_sb[:, tt, :, 1:PH + 1],
                                ps[:, half, 0:HC * PH].rearrange(
                                    "p (h d) -> p h d", h=HC),
                                AF.Copy)
                    chunks.append(emit)
                return chunks

            NB = 8  # kt per batch; QK/AV alternate in batches of NB so PE
            #         tiling-mode switches happen a few times per combo,
            #         not per kt

            class Combo:
                """One (pair, q-group) attention unit, emitted in 4-kt
                batches. AV batches lag QK by 2 batch slots so every exp is
                long done when its AV runs (no per-kt stalls for the
                scheduler to fill with mode-thrashing reordering)."""

                def __init__(self, p, qg, dve):
                    self.p, self.qg, self.dve = p, qg, dve
                    self.cs = np.s_[qg * QCW:(qg + 1) * QCW]
                    self.av = avpool.tile([128, QCW], F32, tag="av",
                                          name="av")
                    self.pts = {}
                    self.r2a = npool.tile([1, QCW], F32, tag="r2a")
                    self.r2b = npool.tile([1, QCW], F32, tag="r2b")
                    self.lrb = npool.tile([1, QCW], F32, tag="lrb")
                    self.rbca = npool.tile([64, QCW], F32, tag="rbca")
                    self.rbcb = npool.tile([64, QCW], F32, tag="rbcb")

                def qk_batch(self, j):
                    for kt in range(j * NB, (j + 1) * NB):
                        st = stpool.tile([128, 2, QCW], F32, tag="st",
                                         name="st")
                        for hh in range(2):
                            b = hh * 64
                            nc.tensor.matmul(
                                st[:, hh, :],
                                qk_sb[b:b + 64, self.p, 1,
                                      kt * 128:(kt + 1) * 128],
                                qk_sb[b:b + 64, self.p, 0, self.cs],
                                start=True, stop=True,
                                skip_group_check=True)
                        pt = ptpool.tile([128, 2, QCW], BF16, tag="pt",
                                         name="pt")
                        if kt % 16 in self.dve:
                            nc.vector.tensor_scalar(
                                pt[:].bitcast(I16), st[:], SCH_A, SCH_B,
                                OP.mult, OP.add)
                        else:
                            nc.scalar.activation(pt[:], st[:], AF.Exp)
                        self.pts[kt] = pt

                def av_batch(self, j):
                    for kt in range(j * NB, (j + 1) * NB):
                        pt = self.pts.pop(kt)
                        for hh in range(2):
                            nc.tensor.matmul(
                                self.av[hh * 64:(hh + 1) * 64, :],
                                v_sb[:, kt, self.p * 2 + hh, :],
                                pt[:, hh, :],
                                start=(kt == 0), stop=(kt == KT - 1),
                                skip_group_check=True)

                def norm_recips(self):
                    # denominators live in av rows 0 / 64
                    nc.vector.reciprocal_approx_fast(self.r2a[:],
                                                     self.av[0:1, :])
                    nc.vector.tensor_copy(self.lrb[:], self.av[64:65, :])
                    nc.vector.reciprocal_approx_fast(self.r2b[:],
                                                     self.lrb[:])
                    nc.gpsimd.partition_broadcast(self.rbca[:], self.r2a[:])
                    nc.gpsimd.partition_broadcast(self.rbcb[:], self.r2b[:])

                def norm_muls(self):
                    # pad rows: av 49-63/113-127 are exact zeros (V' pad
                    # cols) so outT pads come out zero without any memset
                    nc.vector.tensor_mul(outT_sb[0:64, self.p, self.cs],
                                         self.av[0:64, :], self.rbca[:, :])
                    nc.vector.tensor_mul(outT_sb[64:128, self.p, self.cs],
                                         self.av[64:128, :], self.rbcb[:, :])

            def final_proj_chunks(qg):
                """4 emit-closures, one per 128-token tile."""
                chunks = []
                for tt in range(QCW // 128):
                    def emit(tt=tt):
                        t0 = qg * QCW + tt * 128
                        ysb = ypool.tile([128, DIM], F32, tag="ysb")
                        ps = stpool.tile([128, 2, QCW], F32, tag="st",
                                         name="pjy")
                        for jc in range(2):
                            for p2 in range(NP):
                                nc.tensor.matmul(
                                    ps[:, jc, 0:384],
                                    outT_sb[:, p2, t0:t0 + 128],
                                    wo_sb[:, p2, jc * 384:(jc + 1) * 384],
                                    start=(p2 == 0), stop=(p2 == NP - 1),
                                    skip_group_check=True)
                        yv = ysb[:].rearrange("p (a b) -> p a b", a=2)
                        if tt % 2 == 0:
                            nc.scalar.copy(yv, ps[:, :, 0:384])
                        else:
                            nc.vector.tensor_copy(yv, ps[:, :, 0:384])
                        nc.sync.dma_start(y_d[t0:t0 + 128, :], ysb[:])
                    chunks.append(emit)
                return chunks

            # ---- emission order (scheduling priority) ----
            # Pipeline: combo i's QK batches interleave with combo i-1's
            # trailing AV batches and its own leading AV batches; normalize
            # and final-proj chunks slot between batches so serial chains
            # never head-block the exp stream in the FIFO engine queues.
            # qkproj(0) and qkproj(1) go in the DMA-bound prologue.
            for fn in qkT_proj_chunks(0):
                fn()
            for fn in vproj_chunks():
                fn()
            for fn in qkT_proj_chunks(1):
                fn()

            combo_keys = [(qg, p) for qg in range(NQG) for p in range(NP)]
            final_chunks = {}
            prev = None
            for qg, p in combo_keys:
                proj = (qkT_proj_chunks(p + 2)
                        if qg == 0 and p + 2 < NP else [None] * 4)
                if p == 0:
                    final_chunks[qg - 1] = final_proj_chunks(qg - 1) \
                        if qg >= 1 else None
                cur = Combo(p, qg, dve_slots_qg0 if qg == 0 else dve_slots)

                # mode-group structure per combo:
                # [QK kt0-7] [prev AV kt8-15, norm] [proj] [QK kt8-15]
                # [AV kt0-7, prev muls, final] [proj]
                cur.qk_batch(0)
                if prev:
                    prev.av_batch(1)
                    prev.norm_recips()
                if proj[0]:
                    proj[0]()
                if proj[1]:
                    proj[1]()
                cur.qk_batch(1)
                if prev:
                    prev.norm_muls()
                if final_chunks.get(qg - 1):
                    final_chunks[qg - 1][p]()
                if proj[2]:
                    proj[2]()
                if proj[3]:
                    proj[3]()
                cur.av_batch(0)
                prev = cur

            # tail: last combo's trailing AV, normalize, last final projs
            prev.av_batch(1)
            prev.norm_recips()
            prev.norm_muls()
            for fn in final_proj_chunks(NQG - 1):
                fn()

    nc.compile()
    return nc


# ---------------- host-side sharding ----------------

def host_prep(x, w_in, b_in, w_out, T=2048):
    """Full inputs -> list of 8 per-core input dicts."""
    scale = 1.0 / math.sqrt(PH)
    wr = np.asarray(w_in).reshape(DIM, 16, 3, PH)
    br = np.asarray(b_in).reshape(16, 3, PH)
    wog = np.asarray(w_out)  # (768, 768), row dv = h*48+d
    in_maps = []
    for c in range(8):
        b, g = divmod(c, 2)
        wqk = np.zeros((DIM, NP * 2 * 128), np.float32)
        bqk = np.zeros((128, NP * 2), np.float32)
        wv = np.zeros((DIM, HC * PH), np.float32)
        wo = np.zeros((NP * 128, DIM), np.float32)
        for p in range(NP):
            for hh, base in ((0, 0), (1, 64)):
                gh = g * 8 + p * 2 + hh
                wqk[:, (p * 2) * 128 + base:(p * 2) * 128 + base + PH] = wr[:, gh, 0] * scale
                wqk[:, (p * 2 + 1) * 128 + base:(p * 2 + 1) * 128 + base + PH] = wr[:, gh, 1]
                bqk[base:base + PH, p * 2] = br[gh, 0] * scale
                bqk[base:base + PH, p * 2 + 1] = br[gh, 1]
                wv[:, (p * 2 + hh) * PH:(p * 2 + hh + 1) * PH] = wr[:, gh, 2]
                wo[p * 128 + base + 1:p * 128 + base + 1 + PH, :] = wog[gh * PH:(gh + 1) * PH, :]
        in_maps.append({
            "xt": np.ascontiguousarray(np.asarray(x)[b].T).astype(ml_dtypes.bfloat16),
            "wqk": wqk.astype(ml_dtypes.bfloat16),
            "wv": wv.astype(ml_dtypes.bfloat16),
            "wo": wo.astype(ml_dtypes.bfloat16),
            "bqk": bqk,
        })
    return in_maps


def host_post(results, b_out, b_in, w_out, B=4, T=2048):
    # the V bias contributes bv @ w_out, a per-column constant: add on host
    bv_all = np.asarray(b_in).reshape(16, 3, PH)[:, 2, :].reshape(DIM)
    const = np.asarray(b_out) + bv_all @ np.asarray(w_out)
    out = np.empty((B, T, DIM), np.float32)
    for b in range(B):
        out[b] = results[2 * b]["y"] + results[2 * b + 1]["y"] + const[None, :]
    return out


# ---------------- self-contained kernel() entry point ----------------

_CACHED = {}


def _get_nc():
    if "nc" not in _CACHED:
        _CACHED["nc"] = build_kernel(T=2048, num_devices=8)
    return _CACHED["nc"]


def kernel(x, w_in, b_in, w_out, b_out):
    """Full-input MHA forward on 8 NeuronCores.

    x: (4, 2048, 768) f32; w_in: (768, 2304); b_in: (2304,);
    w_out: (768, 768); b_out: (768,). Returns (4, 2048, 768) f32.
    """
    from concourse.bass_utils import run_bass_kernel_spmd

    x = np.asarray(x, np.float32)
    w_in = np.asarray(w_in, np.float32)
    b_in = np.asarray(b_in, np.float32)
    w_out = np.asarray(w_out, np.float32)
    b_out = np.asarray(b_out, np.float32)

    nc = _get_nc()
    in_maps = host_prep(x, w_in, b_in, w_out, T=2048)
    res = run_bass_kernel_spmd(nc, in_maps, core_ids=list(range(8)))
    return host_post(res.results, b_out, b_in, w_out, B=4, T=2048)


# revision 24
# speedup vs baseline: 1.0063x; 1.0063x over previous
"""Bass/Tile MHA kernel for trn2 — builder + host shard/unshard helpers.

Per-core work (8 cores): core c handles batch b=c//2, head-group g=c%2
(8 of 16 heads). v2 design:

- Q^T/K^T are PAIR-PACKED on partitions: head A dims at rows 0-47
  (pad 48-63 = 0), head B at rows 64-111 (pad 112-127 = 0). The QK^T
  matmuls then contract K=64 per head using 64x64 PE array tiling:
  4 concurrent matmuls per k-tile (2 heads x 2 k-token halves) cover
  128 k-tokens x 512 q in ~216ns — 2x less PE time than padding K to
  128. AV matmuls also run as 4 concurrent 64x64 tiles (2 heads x 2
  token halves); the token halves accumulate into separate PSUM banks
  (av_lo/av_hi) which VectorE sums during the softmax normalize.
  All attention matmuls share one (64,64) tiling mode => no PE drains.
- Projections run as batched (128,128)-mode matmul groups between
  attention sections (mode switches cost ~300ns, so they are batched,
  never interleaved per k-tile).
- exp processes BOTH heads' scores in ONE instruction over a 2-bank
  PSUM tile [128, 2, 512] (halves instruction overhead); split between
  ScalarE true-exp and VectorE Schraudolph fast-exp per-kt (tunable).
- No big zero-memsets: the projection activation writes all 128
  partitions; pad rows get bias 0 on zero wqk columns => exact zeros.

Dataflow (all matmuls bf16 in / fp32 PSUM accumulate):
  qkT[d_h, t]  = w_qk^T x + b      (pair-packed, full-128 activation)
  V[t, d_v]    = x w_v             (ones col 0 gives softmax denoms)
  S^T[k, q]    = (K^T)^T Q^T       (4x 64x64 concurrent tiles)
  P^T          = exp(S^T)          (ScalarE exp / VectorE fast-exp)
  av_lo/hi     = (V|1)^T P^T       (4x 64x64 tiles, token halves)
  outT         = (av_lo+av_hi) * bcast(1/l)
  y[t, j]      = outT^T w_out      (+ b_out and cross-core sum on host)
"""

import math

import numpy as np
import ml_dtypes

import concourse.bass as bass
import concourse.mybir as mybir
import concourse.tile as tile
from concourse import bacc

F32 = mybir.dt.float32
BF16 = mybir.dt.bfloat16
I16 = mybir.dt.int16
AF = mybir.ActivationFunctionType
OP = mybir.AluOpType

DIM = 768
PH = 48
NP = 4          # head pairs per core
HC = 8          # heads per core
NDT = DIM // 128  # 6 contraction tiles for the projections

# Schraudolph fast-exp in bf16 bit space: bits = round(x*128/ln2 + (127*128 - C))
SCH_A = 128.0 / math.log(2.0)
SCH_C = 4.7
# +0.5: the fp32->int16 convert truncates, this re-centers it to round-nearest
SCH_B = 127.0 * 128.0 - SCH_C + 0.5

# Which kt slots (mod 16) run fast-exp on VectorE instead of ScalarE exp.
# qg==0 attention overlaps the next pair's projections (ScalarE busier
# with activation copies), so it gives VectorE a larger share.
DVE_SLOTS_QG0 = frozenset({1, 3, 5, 7, 9, 11, 13, 15})
DVE_SLOTS = frozenset({2, 5, 7, 10, 13, 15})


def build_kernel(T=2048, num_devices=8,
                 dve_slots_qg0=DVE_SLOTS_QG0, dve_slots=DVE_SLOTS):
    """Returns compiled Bacc module."""
    KT = T // 128                 # k-tiles (token tiles)
    QCW = min(512, T)             # q chunk width (one PSUM bank)
    NQG = T // QCW                # q groups, one chunk each

    nc = bacc.Bacc("TRN2", target_bir_lowering=False, debug=False,
                   num_devices=num_devices)

    xt_d = nc.dram_tensor("xt", (DIM, T), BF16, kind="ExternalInput")
    wqk_d = nc.dram_tensor("wqk", (DIM, NP * 2 * 128), BF16, kind="ExternalInput")
    wv_d = nc.dram_tensor("wv", (DIM, HC * PH), BF16, kind="ExternalInput")
    wo_d = nc.dram_tensor("wo", (NP * 128, DIM), BF16, kind="ExternalInput")
    bqk_d = nc.dram_tensor("bqk", (128, NP * 2), F32, kind="ExternalInput")
    y_d = nc.dram_tensor("y", (T, DIM), F32, kind="ExternalOutput")

    with tile.TileContext(nc) as tc:
        with (
            tc.tile_pool(name="const", bufs=1) as cpool,
            tc.tile_pool(name="pt", bufs=20) as ptpool,
            tc.tile_pool(name="norm", bufs=2) as npool,
            tc.tile_pool(name="ysb", bufs=2) as ypool,
            tc.tile_pool(name="st", bufs=3, space="PSUM") as stpool,
            tc.tile_pool(name="av", bufs=2, space="PSUM") as avpool,
        ):
            # ---- persistent SBUF tensors ----
            xt_sb = cpool.tile([128, NDT, T], BF16, tag="xt")
            wqk_sb = cpool.tile([128, NDT, NP * 2 * 128], BF16, tag="wqk")
            wv_sb = cpool.tile([128, NDT, HC * PH], BF16, tag="wv")
            wo_sb = cpool.tile([128, NP, DIM], BF16, tag="wo")
            bqk_sb = cpool.tile([128, NP * 2], F32, tag="bqk")
            # pair-packed Q^T/K^T: rows 0-47 head A (48-63 zero), 64-111
            # head B (112-127 zero); slot 0 = Q^T, slot 1 = K^T
            qk_sb = cpool.tile([128, NP, 2, T], BF16, tag="qk")
            # V' columns per head: 0 = ones (softmax denominator lands on
            # PSUM row 0 / 64 of av_lo+av_hi), 1-48 = V, 49-63 = zero
            v_sb = cpool.tile([128, KT, HC, 64], BF16, tag="v")
            outT_sb = cpool.tile([128, NP, T], BF16, tag="outT")

            # ---- input DMAs ----
            # xt + wqk + bqk first (the first projection chunk needs them),
            # wv/wo later (only needed once attention starts)
            nc.sync.dma_start(bqk_sb[:], bqk_d[:])
            # split by partition halves (full DRAM rows stay contiguous) so
            # transfers spread across more queues and land sooner
            for dt_i in range(NDT):
                for h in range(2):
                    ps_ = np.s_[h * 64:(h + 1) * 64]
                    rs = np.s_[dt_i * 128 + h * 64:dt_i * 128 + (h + 1) * 64]
                    nc.sync.dma_start(xt_sb[ps_, dt_i, :], xt_d[rs, :])
                    nc.sync.dma_start(wqk_sb[ps_, dt_i, :], wqk_d[rs, :])
            for dt_i in range(NDT):
                nc.sync.dma_start(wv_sb[:, dt_i, :], wv_d[dt_i * 128:(dt_i + 1) * 128, :])
            for p in range(NP):
                nc.sync.dma_start(wo_sb[:, p, :], wo_d[p * 128:(p + 1) * 128, :])

            # ones column for the softmax-denominator trick; zero pads keep
            # av rows 49-63 / 113-127 finite (they multiply rbc pads below)
            nc.gpsimd.memset(v_sb[:, :, :, 0:1], 1.0)
            nc.gpsimd.memset(v_sb[:, :, :, PH + 1:64], 0.0)

            def qkT_proj_chunks(p):
                """4 emit-closures, one per (q/k slot, 2-chunk group)."""
                chunks = []
                for sl in range(2):
                    for tc2 in range(T // (2 * QCW)):
                        def emit(sl=sl, tc2=tc2):
                            ps = stpool.tile([128, 2, QCW], F32, tag="st",
                                             name="pjqk")
                            col0 = (p * 2 + sl) * 128
                            for half in range(2):
                                tcI = tc2 * 2 + half
                                for dt_i in range(NDT):
                                    nc.tensor.matmul(
                                        ps[:, half, :],
                                        wqk_sb[:, dt_i, col0:col0 + 128],
                                        xt_sb[:, dt_i, tcI * QCW:(tcI + 1) * QCW],
                                        start=(dt_i == 0), stop=(dt_i == NDT - 1),
                                        skip_group_check=True)
                            t0 = tc2 * 2 * QCW
                            nc.scalar.activation(
                                qk_sb[:, p, sl, t0:t0 + 2 * QCW].rearrange(
                                    "p (a b) -> p a b", a=2),
                                ps[:, :, :], AF.Identity,
                                bias=bqk_sb[:, p * 2 + sl:p * 2 + sl + 1])
                        chunks.append(emit)
                return chunks

            def vproj_chunks():
                """8 emit-closures, one per tt pair."""
                chunks = []
                for tp in range(KT // 2):
                    def emit(tp=tp):
                        ps = stpool.tile([128, 2, QCW], F32, tag="st",
                                         name="pjv")
                        for half in range(2):
                            tt = tp * 2 + half
                            for dt_i in range(NDT):
                                nc.tensor.matmul(
                                    ps[:, half, 0:HC * PH],
                                    xt_sb[:, dt_i, tt * 128:(tt + 1) * 128],
                                    wv_sb[:, dt_i, :],
                                    start=(dt_i == 0), stop=(dt_i == NDT - 1),
                                    skip_group_check=True)
                        for half in range(2):
                            tt = tp * 2 + half
                            nc.scalar.activation(
                                v_sb[:, tt, :, 1:PH + 1],
                                ps[:, half, 0:HC * PH].rearrange(
                                    "p (h d) -> p h d", h=HC),
                                AF.Copy)
                    chunks.append(emit)
                return chunks

            NB = 8  # kt per batch; QK/AV alternate in batches of NB so PE
            #         tiling-mode switches happen a few times per combo,
            #         not per kt

            class Combo:
                """One (pair, q-group) attention unit, emitted in 4-kt
                batches. AV batches lag QK by 2 batch slots so every exp is
                long done when its AV runs (no per-kt stalls for the
                scheduler to fill with mode-thrashing reordering)."""

                def __init__(self, p, qg, dve):
                    self.p, self.qg, self.dve = p, qg, dve
                    self.cs = np.s_[qg * QCW:(qg + 1) * QCW]
                    self.av = avpool.tile([128, QCW], F32, tag="av",
                                          name="av")
                    self.pts = {}
                    self.r2a = npool.tile([1, QCW], F32, tag="r2a")
                    self.r2b = npool.tile([1, QCW], F32, tag="r2b")
                    self.lrb = npool.tile([1, QCW], F32, tag="lrb")
                    self.rbca = npool.tile([64, QCW], F32, tag="rbca")
                    self.rbcb = npool.tile([64, QCW], F32, tag="rbcb")

                def qk_batch(self, j):
                    for kt in range(j * NB, (j + 1) * NB):
                        st = stpool.tile([128, 2, QCW], F32, tag="st",
                                         name="st")
                        for hh in range(2):
                            b = hh * 64
                            nc.tensor.matmul(
                                st[:, hh, :],
                                qk_sb[b:b + 64, self.p, 1,
                                      kt * 128:(kt + 1) * 128],
                                qk_sb[b:b + 64, self.p, 0, self.cs],
                                start=True, stop=True,
                                skip_group_check=True)
                        pt = ptpool.tile([128, 2, QCW], BF16, tag="pt",
                                         name="pt")
                        if kt % 16 in self.dve:
                            nc.vector.tensor_scalar(
                                pt[:].bitcast(I16), st[:], SCH_A, SCH_B,
                                OP.mult, OP.add)
                        else:
                            nc.scalar.activation(pt[:], st[:], AF.Exp)
                        self.pts[kt] = pt

                def av_batch(self, j):
                    for kt in range(j * NB, (j + 1) * NB):
                        pt = self.pts.pop(kt)
                        for hh in range(2):
                            nc.tensor.matmul(
                                self.av[hh * 64:(hh + 1) * 64, :],
                                v_sb[:, kt, self.p * 2 + hh, :],
                                pt[:, hh, :],
                                start=(kt == 0), stop=(kt == KT - 1),
                                skip_group_check=True)

                def norm_recips(self):
                    # denominators live in av rows 0 / 64
                    nc.vector.reciprocal_approx_fast(self.r2a[:],
                                                     self.av[0:1, :])
                    nc.vector.tensor_copy(self.lrb[:], self.av[64:65, :])
                    nc.vector.reciprocal_approx_fast(self.r2b[:],
                                                     self.lrb[:])
                    nc.gpsimd.partition_broadcast(self.rbca[:], self.r2a[:])
                    nc.gpsimd.partition_broadcast(self.rbcb[:], self.r2b[:])

                def norm_muls(self):
                    # pad rows: av 49-63/113-127 are exact zeros (V' pad
                    # cols) so outT pads come out zero without any memset
                    nc.vector.tensor_mul(outT_sb[0:64, self.p, self.cs],
                                         self.av[0:64, :], self.rbca[:, :])
                    nc.vector.tensor_mul(outT_sb[64:128, self.p, self.cs],
                                         self.av[64:128, :], self.rbcb[:, :])

            def final_proj_chunks(qg):
                """4 emit-closures, one per 128-token tile."""
                chunks = []
                for tt in range(QCW // 128):
                    def emit(tt=tt):
                        t0 = qg * QCW + tt * 128
                        ysb = ypool.tile([128, DIM], F32, tag="ysb")
                        ps = stpool.tile([128, 2, QCW], F32, tag="st",
                                         name="pjy")
                        for jc in range(2):
                            for p2 in range(NP):
                                nc.tensor.matmul(
                                    ps[:, jc, 0:384],
                                    outT_sb[:, p2, t0:t0 + 128],
                                    wo_sb[:, p2, jc * 384:(jc + 1) * 384],
                                    start=(p2 == 0), stop=(p2 == NP - 1),
                                    skip_group_check=True)
                        yv = ysb[:].rearrange("p (a b) -> p a b", a=2)
                        if tt % 2 == 0:
                            nc.scalar.copy(yv, ps[:, :, 0:384])
                        else:
                            nc.vector.tensor_copy(yv, ps[:, :, 0:384])
                        nc.sync.dma_start(y_d[t0:t0 + 128, :], ysb[:])
                    chunks.append(emit)
                return chunks

            # ---- emission order (scheduling priority) ----
            # Pipeline: combo i's QK batches interleave with combo i-1's
            # trailing AV batches and its own leading AV batches; normalize
            # and final-proj chunks slot between batches so serial chains
            # never head-block the exp stream in the FIFO engine queues.
            # qkproj(0) and qkproj(1) go in the DMA-bound prologue.
            for fn in qkT_proj_chunks(0):
                fn()
            for fn in vproj_chunks():
                fn()
            for fn in qkT_proj_chunks(1):
                fn()

            combo_keys = [(qg, p) for qg in range(NQG) for p in range(NP)]
            final_chunks = {}
            prev = None
            for qg, p in combo_keys:
                proj = (qkT_proj_chunks(p + 2)
                        if qg == 0 and p + 2 < NP else [None] * 4)
                if p == 0:
                    final_chunks[qg - 1] = final_proj_chunks(qg - 1) \
                        if qg >= 1 else None
                cur = Combo(p, qg, dve_slots_qg0 if qg == 0 else dve_slots)

                # mode-group structure per combo:
                # [QK kt0-7] [prev AV kt8-15, norm] [proj] [QK kt8-15]
                # [AV kt0-7, prev muls, final] [proj]
                cur.qk_batch(0)
                if prev:
                    prev.av_batch(1)
                    prev.norm_recips()
                if proj[0]:
                    proj[0]()
                if proj[1]:
                    proj[1]()
                cur.qk_batch(1)
                cur.av_batch(0)
                if prev:
                    prev.norm_muls()
                if final_chunks.get(qg - 1):
                    final_chunks[qg - 1][p]()
                if proj[2]:
                    proj[2]()
                if proj[3]:
                    proj[3]()
                prev = cur

            # tail: last combo's trailing AV, normalize, last final projs
            prev.av_batch(1)
            prev.norm_recips()
            prev.norm_muls()
            for fn in final_proj_chunks(NQG - 1):
                fn()

    nc.compile()
    return nc


# ---------------- host-side sharding ----------------

def host_prep(x, w_in, b_in, w_out, T=2048):
    """Full inputs -> list of 8 per-core input dicts."""
    scale = 1.0 / math.sqrt(PH)
    wr = np.asarray(w_in).reshape(DIM, 16, 3, PH)
    br = np.asarray(b_in).reshape(16, 3, PH)
    wog = np.asarray(w_out)  # (768, 768), row dv = h*48+d
    in_maps = []
    for c in range(8):
        b, g = divmod(c, 2)
        wqk = np.zeros((DIM, NP * 2 * 128), np.float32)
        bqk = np.zeros((128, NP * 2), np.float32)
        wv = np.zeros((DIM, HC * PH), np.float32)
        wo = np.zeros((NP * 128, DIM), np.float32)
        for p in range(NP):
            for hh, base in ((0, 0), (1, 64)):
                gh = g * 8 + p * 2 + hh
                wqk[:, (p * 2) * 128 + base:(p * 2) * 128 + base + PH] = wr[:, gh, 0] * scale
                wqk[:, (p * 2 + 1) * 128 + base:(p * 2 + 1) * 128 + base + PH] = wr[:, gh, 1]
                bqk[base:base + PH, p * 2] = br[gh, 0] * scale
                bqk[base:base + PH, p * 2 + 1] = br[gh, 1]
                wv[:, (p * 2 + hh) * PH:(p * 2 + hh + 1) * PH] = wr[:, gh, 2]
                wo[p * 128 + base + 1:p * 128 + base + 1 + PH, :] = wog[gh * PH:(gh + 1) * PH, :]
        in_maps.append({
            "xt": np.ascontiguousarray(np.asarray(x)[b].T).astype(ml_dtypes.bfloat16),
            "wqk": wqk.astype(ml_dtypes.bfloat16),
            "wv": wv.astype(ml_dtypes.bfloat16),
            "wo": wo.astype(ml_dtypes.bfloat16),
            "bqk": bqk,
        })
    return in_maps


def host_post(results, b_out, b_in, w_out, B=4, T=2048):
    # the V bias contributes bv @ w_out, a per-column constant: add on host
    bv_all = np.asarray(b_in).reshape(16, 3, PH)[:, 2, :].reshape(DIM)
    const = np.asarray(b_out) + bv_all @ np.asarray(w_out)
    out = np.empty((B, T, DIM), np.float32)
    for b in range(B):
        out[b] = results[2 * b]["y"] + results[2 * b + 1]["y"] + const[None, :]
    return out


# ---------------- self-contained kernel() entry point ----------------

_CACHED = {}


def _get_nc():
    if "nc" not in _CACHED:
        _CACHED["nc"] = build_kernel(T=2048, num_devices=8)
    return _CACHED["nc"]


def kernel(x, w_in, b_in, w_out, b_out):
    """Full-input MHA forward on 8 NeuronCores.

    x: (4, 2048, 768) f32; w_in: (768, 2304); b_in: (2304,);
    w_out: (768, 768); b_out: (768,). Returns (4, 2048, 768) f32.
    """
    from concourse.bass_utils import run_bass_kernel_spmd

    x = np.asarray(x, np.float32)
    w_in = np.asarray(w_in, np.float32)
    b_in = np.asarray(b_in, np.float32)
    w_out = np.asarray(w_out, np.float32)
    b_out = np.asarray(b_out, np.float32)

    nc = _get_nc()
    in_maps = host_prep(x, w_in, b_in, w_out, T=2048)
    res = run_bass_kernel_spmd(nc, in_maps, core_ids=list(range(8)))
    return host_post(res.results, b_out, b_in, w_out, B=4, T=2048)


# revision 27
# speedup vs baseline: 1.0196x; 1.0132x over previous
"""Bass/Tile MHA kernel for trn2 — builder + host shard/unshard helpers.

Per-core work (8 cores): core c handles batch b=c//2, head-group g=c%2
(8 of 16 heads). v2 design:

- Q^T/K^T are PAIR-PACKED on partitions: head A dims at rows 0-47
  (pad 48-63 = 0), head B at rows 64-111 (pad 112-127 = 0). The QK^T
  matmuls then contract K=64 per head using 64x64 PE array tiling:
  4 concurrent matmuls per k-tile (2 heads x 2 k-token halves) cover
  128 k-tokens x 512 q in ~216ns — 2x less PE time than padding K to
  128. AV matmuls also run as 4 concurrent 64x64 tiles (2 heads x 2
  token halves); the token halves accumulate into separate PSUM banks
  (av_lo/av_hi) which VectorE sums during the softmax normalize.
  All attention matmuls share one (64,64) tiling mode => no PE drains.
- Projections run as batched (128,128)-mode matmul groups between
  attention sections (mode switches cost ~300ns, so they are batched,
  never interleaved per k-tile).
- exp processes BOTH heads' scores in ONE instruction over a 2-bank
  PSUM tile [128, 2, 512] (halves instruction overhead); split between
  ScalarE true-exp and VectorE Schraudolph fast-exp per-kt (tunable).
- No big zero-memsets: the projection activation writes all 128
  partitions; pad rows get bias 0 on zero wqk columns => exact zeros.

Dataflow (all matmuls bf16 in / fp32 PSUM accumulate):
  qkT[d_h, t]  = w_qk^T x + b      (pair-packed, full-128 activation)
  V[t, d_v]    = x w_v             (ones col 0 gives softmax denoms)
  S^T[k, q]    = (K^T)^T Q^T       (4x 64x64 concurrent tiles)
  P^T          = exp(S^T)          (ScalarE exp / VectorE fast-exp)
  av_lo/hi     = (V|1)^T P^T       (4x 64x64 tiles, token halves)
  outT         = (av_lo+av_hi) * bcast(1/l)
  y[t, j]      = outT^T w_out      (+ b_out and cross-core sum on host)
"""

import math

import numpy as np
import ml_dtypes

import concourse.bass as bass
import concourse.mybir as mybir
import concourse.tile as tile
from concourse import bacc

F32 = mybir.dt.float32
BF16 = mybir.dt.bfloat16
I16 = mybir.dt.int16
AF = mybir.ActivationFunctionType
OP = mybir.AluOpType

DIM = 768
PH = 48
NP = 4          # head pairs per core
HC = 8          # heads per core
NDT = DIM // 128  # 6 contraction tiles for the projections

# Schraudolph fast-exp in bf16 bit space: bits = round(x*128/ln2 + (127*128 - C))
SCH_A = 128.0 / math.log(2.0)
SCH_C = 4.7
# +0.5: the fp32->int16 convert truncates, this re-centers it to round-nearest
SCH_B = 127.0 * 128.0 - SCH_C + 0.5

# Which kt slots (mod 16) run fast-exp on VectorE instead of ScalarE exp.
# qg==0 attention overlaps the next pair's projections (ScalarE busier
# with activation copies), so it gives VectorE a larger share.
DVE_SLOTS_QG0 = frozenset({1, 3, 5, 7, 9, 11, 13, 15})
DVE_SLOTS = frozenset({2, 5, 7, 10, 13, 15})


def build_kernel(T=2048, num_devices=8,
                 dve_slots_qg0=DVE_SLOTS_QG0, dve_slots=DVE_SLOTS):
    """Returns compiled Bacc module."""
    KT = T // 128                 # k-tiles (token tiles)
    QCW = min(512, T)             # q chunk width (one PSUM bank)
    NQG = T // QCW                # q groups, one chunk each

    nc = bacc.Bacc("TRN2", target_bir_lowering=False, debug=False,
                   num_devices=num_devices)

    xt_d = nc.dram_tensor("xt", (DIM, T), BF16, kind="ExternalInput")
    wqk_d = nc.dram_tensor("wqk", (DIM, NP * 2 * 128), BF16, kind="ExternalInput")
    wv_d = nc.dram_tensor("wv", (DIM, HC * PH), BF16, kind="ExternalInput")
    wo_d = nc.dram_tensor("wo", (NP * 128, DIM), BF16, kind="ExternalInput")
    bqk_d = nc.dram_tensor("bqk", (128, NP * 2), F32, kind="ExternalInput")
    y_d = nc.dram_tensor("y", (T, DIM), F32, kind="ExternalOutput")

    with tile.TileContext(nc) as tc:
        with (
            tc.tile_pool(name="const", bufs=1) as cpool,
            tc.tile_pool(name="pt", bufs=20) as ptpool,
            tc.tile_pool(name="norm", bufs=2) as npool,
            tc.tile_pool(name="ysb", bufs=2) as ypool,
            tc.tile_pool(name="st", bufs=3, space="PSUM") as stpool,
            tc.tile_pool(name="av", bufs=2, space="PSUM") as avpool,
        ):
            # ---- persistent SBUF tensors ----
            xt_sb = cpool.tile([128, NDT, T], BF16, tag="xt")
            wqk_sb = cpool.tile([128, NDT, NP * 2 * 128], BF16, tag="wqk")
            wv_sb = cpool.tile([128, NDT, HC * PH], BF16, tag="wv")
            wo_sb = cpool.tile([128, NP, DIM], BF16, tag="wo")
            bqk_sb = cpool.tile([128, NP * 2], F32, tag="bqk")
            # pair-packed Q^T/K^T: rows 0-47 head A (48-63 zero), 64-111
            # head B (112-127 zero); slot 0 = Q^T, slot 1 = K^T
            qk_sb = cpool.tile([128, NP, 2, T], BF16, tag="qk")
            # V' columns per head: 0 = ones (softmax denominator lands on
            # PSUM row 0 / 64 of av_lo+av_hi), 1-48 = V, 49-63 = zero
            v_sb = cpool.tile([128, KT, HC, 64], BF16, tag="v")
            outT_sb = cpool.tile([128, NP, T], BF16, tag="outT")

            # ---- input DMAs ----
            # xt + wqk + bqk first (the first projection chunk needs them),
            # wv/wo later (only needed once attention starts)
            nc.sync.dma_start(bqk_sb[:], bqk_d[:])
            for dt_i in range(NDT):
                nc.sync.dma_start(xt_sb[:, dt_i, :], xt_d[dt_i * 128:(dt_i + 1) * 128, :])
                nc.sync.dma_start(wqk_sb[:, dt_i, :], wqk_d[dt_i * 128:(dt_i + 1) * 128, :])
            for dt_i in range(NDT):
                nc.sync.dma_start(wv_sb[:, dt_i, :], wv_d[dt_i * 128:(dt_i + 1) * 128, :])
            for p in range(NP):
                nc.sync.dma_start(wo_sb[:, p, :], wo_d[p * 128:(p + 1) * 128, :])

            # ones column for the softmax-denominator trick; zero pads keep
            # av rows 49-63 / 113-127 finite (they multiply rbc pads below)
            nc.gpsimd.memset(v_sb[:, :, :, 0:1], 1.0)
            nc.gpsimd.memset(v_sb[:, :, :, PH + 1:64], 0.0)

            def qkT_proj_chunks(p):
                """4 emit-closures, one per (q/k slot, 2-chunk group)."""
                chunks = []
                for sl in range(2):
                    for tc2 in range(T // (2 * QCW)):
                        def emit(sl=sl, tc2=tc2):
                            ps = stpool.tile([128, 2, QCW], F32, tag="st",
                                             name="pjqk")
                            col0 = (p * 2 + sl) * 128
                            for half in range(2):
                                tcI = tc2 * 2 + half
                                for dt_i in range(NDT):
                                    nc.tensor.matmul(
                                        ps[:, half, :],
                                        wqk_sb[:, dt_i, col0:col0 + 128],
                                        xt_sb[:, dt_i, tcI * QCW:(tcI + 1) * QCW],
                                        start=(dt_i == 0), stop=(dt_i == NDT - 1),
                                        skip_group_check=True)
                            t0 = tc2 * 2 * QCW
                            nc.scalar.activation(
                                qk_sb[:, p, sl, t0:t0 + 2 * QCW].rearrange(
                                    "p (a b) -> p a b", a=2),
                                ps[:, :, :], AF.Identity,
                                bias=bqk_sb[:, p * 2 + sl:p * 2 + sl + 1])
                        chunks.append(emit)
                return chunks

            def vproj_chunks():
                """8 emit-closures, one per tt pair."""
                chunks = []
                for tp in range(KT // 2):
                    def emit(tp=tp):
                        ps = stpool.tile([128, 2, QCW], F32, tag="st",
                                         name="pjv")
                        for half in range(2):
                            tt = tp * 2 + half
                            for dt_i in range(NDT):
                                nc.tensor.matmul(
                                    ps[:, half, 0:HC * PH],
                                    xt_sb[:, dt_i, tt * 128:(tt + 1) * 128],
                                    wv_sb[:, dt_i, :],
                                    start=(dt_i == 0), stop=(dt_i == NDT - 1),
                                    skip_group_check=True)
                        for half in range(2):
                            tt = tp * 2 + half
                            nc.scalar.activation(
                                v_sb[:, tt, :, 1:PH + 1],
                                ps[:, half, 0:HC * PH].rearrange(
                                    "p (h d) -> p h d", h=HC),
                                AF.Copy)
                    chunks.append(emit)
                return chunks

            NB = 8  # kt per batch; QK/AV alternate in batches of NB so PE
            #         tiling-mode switches happen a few times per combo,
            #         not per kt

            class Combo:
                """One (pair, q-group) attention unit, emitted in 4-kt
                batches. AV batches lag QK by 2 batch slots so every exp is
                long done when its AV runs (no per-kt stalls for the
                scheduler to fill with mode-thrashing reordering)."""

                def __init__(self, p, qg, dve):
                    self.p, self.qg, self.dve = p, qg, dve
                    self.cs = np.s_[qg * QCW:(qg + 1) * QCW]
                    self.av = avpool.tile([128, QCW], F32, tag="av",
                                          name="av")
                    self.pts = {}
                    self.r2a = npool.tile([1, QCW], F32, tag="r2a")
                    self.r2b = npool.tile([1, QCW], F32, tag="r2b")
                    self.lrb = npool.tile([1, QCW], F32, tag="lrb")
                    self.rbca = npool.tile([64, QCW], F32, tag="rbca")
                    self.rbcb = npool.tile([64, QCW], F32, tag="rbcb")

                def qk_batch(self, j):
                    for kt in range(j * NB, (j + 1) * NB):
                        st = stpool.tile([128, 2, QCW], F32, tag="st",
                                         name="st")
                        for hh in range(2):
                            b = hh * 64
                            nc.tensor.matmul(
                                st[:, hh, :],
                                qk_sb[b:b + 64, self.p, 1,
                                      kt * 128:(kt + 1) * 128],
                                qk_sb[b:b + 64, self.p, 0, self.cs],
                                start=True, stop=True,
                                skip_group_check=True)
                        pt = ptpool.tile([128, 2, QCW], BF16, tag="pt",
                                         name="pt")
                        if kt % 16 in self.dve:
                            nc.vector.tensor_scalar(
                                pt[:].bitcast(I16), st[:], SCH_A, SCH_B,
                                OP.mult, OP.add)
                        else:
                            nc.scalar.activation(pt[:], st[:], AF.Exp)
                        self.pts[kt] = pt

                def av_batch(self, j):
                    for kt in range(j * NB, (j + 1) * NB):
                        pt = self.pts.pop(kt)
                        for hh in range(2):
                            nc.tensor.matmul(
                                self.av[hh * 64:(hh + 1) * 64, :],
                                v_sb[:, kt, self.p * 2 + hh, :],
                                pt[:, hh, :],
                                start=(kt == 0), stop=(kt == KT - 1),
                                skip_group_check=True)

                def norm_recips(self):
                    # denominators live in av rows 0 / 64
                    nc.vector.reciprocal_approx_fast(self.r2a[:],
                                                     self.av[0:1, :])
                    nc.vector.tensor_copy(self.lrb[:], self.av[64:65, :])
                    nc.vector.reciprocal_approx_fast(self.r2b[:],
                                                     self.lrb[:])
                    nc.gpsimd.partition_broadcast(self.rbca[:], self.r2a[:])
                    nc.gpsimd.partition_broadcast(self.rbcb[:], self.r2b[:])

                def norm_muls(self):
                    # pad rows: av 49-63/113-127 are exact zeros (V' pad
                    # cols) so outT pads come out zero without any memset
                    nc.vector.tensor_mul(outT_sb[0:64, self.p, self.cs],
                                         self.av[0:64, :], self.rbca[:, :])
                    nc.vector.tensor_mul(outT_sb[64:128, self.p, self.cs],
                                         self.av[64:128, :], self.rbcb[:, :])

            def final_proj_chunks(qg):
                """4 emit-closures, one per 128-token tile."""
                chunks = []
                for tt in range(QCW // 128):
                    def emit(tt=tt):
                        t0 = qg * QCW + tt * 128
                        ysb = ypool.tile([128, DIM], F32, tag="ysb")
                        ps = stpool.tile([128, 2, QCW], F32, tag="st",
                                         name="pjy")
                        for jc in range(2):
                            for p2 in range(NP):
                                nc.tensor.matmul(
                                    ps[:, jc, 0:384],
                                    outT_sb[:, p2, t0:t0 + 128],
                                    wo_sb[:, p2, jc * 384:(jc + 1) * 384],
                                    start=(p2 == 0), stop=(p2 == NP - 1),
                                    skip_group_check=True)
                        yv = ysb[:].rearrange("p (a b) -> p a b", a=2)
                        if tt % 2 == 0:
                            nc.scalar.copy(yv, ps[:, :, 0:384])
                        else:
                            nc.vector.tensor_copy(yv, ps[:, :, 0:384])
                        nc.sync.dma_start(y_d[t0:t0 + 128, :], ysb[:])
                    chunks.append(emit)
                return chunks

            # ---- emission order (scheduling priority) ----
            # Pipeline: combo i's QK batches interleave with combo i-1's
            # trailing AV batches and its own leading AV batches; normalize
            # and final-proj chunks slot between batches so serial chains
            # never head-block the exp stream in the FIFO engine queues.
            # Only qkproj(0) runs in the DMA-bound prologue; vproj and the
            # other qkproj pairs overlap the first combos' exp streams.
            for fn in qkT_proj_chunks(0):
                fn()

            vch = vproj_chunks()
            qkp = {pp: qkT_proj_chunks(pp) for pp in (1, 2, 3)}
            # pre: before prev's trailing AV batch; mid: between QK
            # batches; late: after the leading AV batch
            pre_inj = {1: vch[4:8]}
            mid_inj = {0: vch[0:4] + qkp[1][0:2],
                       1: qkp[2][0:2],
                       2: qkp[3][0:2]}
            late_inj = {0: qkp[1][2:4],
                        1: qkp[2][2:4],
                        2: qkp[3][2:4]}
            combo_keys = [(qg, p) for qg in range(NQG) for p in range(NP)]
            final_chunks = {}
            prev = None
            for i, (qg, p) in enumerate(combo_keys):
                if p == 0:
                    final_chunks[qg - 1] = final_proj_chunks(qg - 1) \
                        if qg >= 1 else None
                cur = Combo(p, qg, dve_slots_qg0 if qg == 0 else dve_slots)

                # mode-group structure per combo:
                # [QK kt0-7] [prev AV kt8-15, norm] [proj] [QK kt8-15]
                # [AV kt0-7, prev muls, final] [proj]
                cur.qk_batch(0)
                for fn in pre_inj.get(i, []):
                    fn()
                if prev:
                    prev.av_batch(1)
                    prev.norm_recips()
                for fn in mid_inj.get(i, []):
                    fn()
                cur.qk_batch(1)
                cur.av_batch(0)
                if prev:
                    prev.norm_muls()
                if final_chunks.get(qg - 1):
                    final_chunks[qg - 1][p]()
                for fn in late_inj.get(i, []):
                    fn()
                prev = cur

            # tail: last combo's trailing AV, normalize, last final projs
            prev.av_batch(1)
            prev.norm_recips()
            prev.norm_muls()
            for fn in final_proj_chunks(NQG - 1):
                fn()

    nc.compile()
    return nc


# ---------------- host-side sharding ----------------

def host_prep(x, w_in, b_in, w_out, T=2048):
    """Full inputs -> list of 8 per-core input dicts."""
    scale = 1.0 / math.sqrt(PH)
    wr = np.asarray(w_in).reshape(DIM, 16, 3, PH)
    br = np.asarray(b_in).reshape(16, 3, PH)
    wog = np.asarray(w_out)  # (768, 768), row dv = h*48+d
    in_maps = []
    for c in range(8):
        b, g = divmod(c, 2)
        wqk = np.zeros((DIM, NP * 2 * 128), np.float32)
        bqk = np.zeros((128, NP * 2), np.float32)
        wv = np.zeros((DIM, HC * PH), np.float32)
        wo = np.zeros((NP * 128, DIM), np.float32)
        for p in range(NP):
            for hh, base in ((0, 0), (1, 64)):
                gh = g * 8 + p * 2 + hh
                wqk[:, (p * 2) * 128 + base:(p * 2) * 128 + base + PH] = wr[:, gh, 0] * scale
                wqk[:, (p * 2 + 1) * 128 + base:(p * 2 + 1) * 128 + base + PH] = wr[:, gh, 1]
                bqk[base:base + PH, p * 2] = br[gh, 0] * scale
                bqk[base:base + PH, p * 2 + 1] = br[gh, 1]
                wv[:, (p * 2 + hh) * PH:(p * 2 + hh + 1) * PH] = wr[:, gh, 2]
                wo[p * 128 + base + 1:p * 128 + base + 1 + PH, :] = wog[gh * PH:(gh + 1) * PH, :]
        in_maps.append({
            "xt": np.ascontiguousarray(np.asarray(x)[b].T).astype(ml_dtypes.bfloat16),
            "wqk": wqk.astype(ml_dtypes.bfloat16),
            "wv": wv.astype(ml_dtypes.bfloat16),
            "wo": wo.astype(ml_dtypes.bfloat16),
            "bqk": bqk,
        })
    return in_maps


def host_post(results, b_out, b_in, w_out, B=4, T=2048):
    # the V bias contributes bv @ w_out, a per-column constant: add on host
    bv_all = np.asarray(b_in).reshape(16, 3, PH)[:, 2, :].reshape(DIM)
    const = np.asarray(b_out) + bv_all @ np.asarray(w_out)
    out = np.empty((B, T, DIM), np.float32)
    for b in range(B):
        out[b] = results[2 * b]["y"] + results[2 * b + 1]["y"] + const[None, :]
    return out


# ---------------- self-contained kernel() entry point ----------------

_CACHED = {}


def _get_nc():
    if "nc" not in _CACHED:
        _CACHED["nc"] = build_kernel(T=2048, num_devices=8)
    return _CACHED["nc"]


def kernel(x, w_in, b_in, w_out, b_out):
    """Full-input MHA forward on 8 NeuronCores.

    x: (4, 2048, 768) f32; w_in: (768, 2304); b_in: (2304,);
    w_out: (768, 768); b_out: (768,). Returns (4, 2048, 768) f32.
    """
    from concourse.bass_utils import run_bass_kernel_spmd

    x = np.asarray(x, np.float32)
    w_in = np.asarray(w_in, np.float32)
    b_in = np.asarray(b_in, np.float32)
    w_out = np.asarray(w_out, np.float32)
    b_out = np.asarray(b_out, np.float32)

    nc = _get_nc()
    in_maps = host_prep(x, w_in, b_in, w_out, T=2048)
    res = run_bass_kernel_spmd(nc, in_maps, core_ids=list(range(8)))
    return host_post(res.results, b_out, b_in, w_out, B=4, T=2048)


# revision 28
# speedup vs baseline: 1.0267x; 1.0069x over previous
"""Bass/Tile MHA kernel for trn2 — builder + host shard/unshard helpers.

Per-core work (8 cores): core c handles batch b=c//2, head-group g=c%2
(8 of 16 heads). v2 design:

- Q^T/K^T are PAIR-PACKED on partitions: head A dims at rows 0-47
  (pad 48-63 = 0), head B at rows 64-111 (pad 112-127 = 0). The QK^T
  matmuls then contract K=64 per head using 64x64 PE array tiling:
  4 concurrent matmuls per k-tile (2 heads x 2 k-token halves) cover
  128 k-tokens x 512 q in ~216ns — 2x less PE time than padding K to
  128. AV matmuls also run as 4 concurrent 64x64 tiles (2 heads x 2
  token halves); the token halves accumulate into separate PSUM banks
  (av_lo/av_hi) which VectorE sums during the softmax normalize.
  All attention matmuls share one (64,64) tiling mode => no PE drains.
- Projections run as batched (128,128)-mode matmul groups between
  attention sections (mode switches cost ~300ns, so they are batched,
  never interleaved per k-tile).
- exp processes BOTH heads' scores in ONE instruction over a 2-bank
  PSUM tile [128, 2, 512] (halves instruction overhead); split between
  ScalarE true-exp and VectorE Schraudolph fast-exp per-kt (tunable).
- No big zero-memsets: the projection activation writes all 128
  partitions; pad rows get bias 0 on zero wqk columns => exact zeros.

Dataflow (all matmuls bf16 in / fp32 PSUM accumulate):
  qkT[d_h, t]  = w_qk^T x + b      (pair-packed, full-128 activation)
  V[t, d_v]    = x w_v             (ones col 0 gives softmax denoms)
  S^T[k, q]    = (K^T)^T Q^T       (4x 64x64 concurrent tiles)
  P^T          = exp(S^T)          (ScalarE exp / VectorE fast-exp)
  av_lo/hi     = (V|1)^T P^T       (4x 64x64 tiles, token halves)
  outT         = (av_lo+av_hi) * bcast(1/l)
  y[t, j]      = outT^T w_out      (+ b_out and cross-core sum on host)
"""

import math

import numpy as np
import ml_dtypes

import concourse.bass as bass
import concourse.mybir as mybir
import concourse.tile as tile
from concourse import bacc

F32 = mybir.dt.float32
BF16 = mybir.dt.bfloat16
I16 = mybir.dt.int16
AF = mybir.ActivationFunctionType
OP = mybir.AluOpType

DIM = 768
PH = 48
NP = 4          # head pairs per core
HC = 8          # heads per core
NDT = DIM // 128  # 6 contraction tiles for the projections

# Schraudolph fast-exp in bf16 bit space: bits = round(x*128/ln2 + (127*128 - C))
SCH_A = 128.0 / math.log(2.0)
SCH_C = 4.7
# +0.5: the fp32->int16 convert truncates, this re-centers it to round-nearest
SCH_B = 127.0 * 128.0 - SCH_C + 0.5

# Which kt slots (mod 16) run fast-exp on VectorE instead of ScalarE exp.
# qg==0 attention overlaps the next pair's projections (ScalarE busier
# with activation copies), so it gives VectorE a larger share.
DVE_SLOTS_QG0 = frozenset({1, 3, 5, 7, 9, 11, 13, 15})
DVE_SLOTS = frozenset({2, 5, 7, 10, 13, 15})


def build_kernel(T=2048, num_devices=8,
                 dve_slots_qg0=DVE_SLOTS_QG0, dve_slots=DVE_SLOTS):
    """Returns compiled Bacc module."""
    KT = T // 128                 # k-tiles (token tiles)
    QCW = min(512, T)             # q chunk width (one PSUM bank)
    NQG = T // QCW                # q groups, one chunk each

    nc = bacc.Bacc("TRN2", target_bir_lowering=False, debug=False,
                   num_devices=num_devices)

    xt_d = nc.dram_tensor("xt", (DIM, T), BF16, kind="ExternalInput")
    wqk_d = nc.dram_tensor("wqk", (DIM, NP * 2 * 128), BF16, kind="ExternalInput")
    wv_d = nc.dram_tensor("wv", (DIM, HC * PH), BF16, kind="ExternalInput")
    wo_d = nc.dram_tensor("wo", (NP * 128, DIM), BF16, kind="ExternalInput")
    bqk_d = nc.dram_tensor("bqk", (128, NP * 2), F32, kind="ExternalInput")
    y_d = nc.dram_tensor("y", (T, DIM), F32, kind="ExternalOutput")

    with tile.TileContext(nc) as tc:
        with (
            tc.tile_pool(name="const", bufs=1) as cpool,
            tc.tile_pool(name="pt", bufs=20) as ptpool,
            tc.tile_pool(name="norm", bufs=2) as npool,
            tc.tile_pool(name="ysb", bufs=2) as ypool,
            tc.tile_pool(name="st", bufs=3, space="PSUM") as stpool,
            tc.tile_pool(name="av", bufs=2, space="PSUM") as avpool,
        ):
            # ---- persistent SBUF tensors ----
            xt_sb = cpool.tile([128, NDT, T], BF16, tag="xt")
            wqk_sb = cpool.tile([128, NDT, NP * 2 * 128], BF16, tag="wqk")
            wv_sb = cpool.tile([128, NDT, HC * PH], BF16, tag="wv")
            wo_sb = cpool.tile([128, NP, DIM], BF16, tag="wo")
            bqk_sb = cpool.tile([128, NP * 2], F32, tag="bqk")
            # pair-packed Q^T/K^T: rows 0-47 head A (48-63 zero), 64-111
            # head B (112-127 zero); slot 0 = Q^T, slot 1 = K^T
            qk_sb = cpool.tile([128, NP, 2, T], BF16, tag="qk")
            # V' columns per head: 0 = ones (softmax denominator lands on
            # PSUM row 0 / 64 of av_lo+av_hi), 1-48 = V, 49-63 = zero
            v_sb = cpool.tile([128, KT, HC, 64], BF16, tag="v")
            outT_sb = cpool.tile([128, NP, T], BF16, tag="outT")

            # ---- input DMAs ----
            # xt + wqk + bqk first (the first projection chunk needs them),
            # wv/wo later (only needed once attention starts)
            nc.sync.dma_start(bqk_sb[:], bqk_d[:])
            for dt_i in range(NDT):
                nc.sync.dma_start(xt_sb[:, dt_i, :], xt_d[dt_i * 128:(dt_i + 1) * 128, :])
                nc.sync.dma_start(wqk_sb[:, dt_i, :], wqk_d[dt_i * 128:(dt_i + 1) * 128, :])
            for dt_i in range(NDT):
                nc.sync.dma_start(wv_sb[:, dt_i, :], wv_d[dt_i * 128:(dt_i + 1) * 128, :])
            for p in range(NP):
                nc.sync.dma_start(wo_sb[:, p, :], wo_d[p * 128:(p + 1) * 128, :])

            # ones column for the softmax-denominator trick; zero pads keep
            # av rows 49-63 / 113-127 finite (they multiply rbc pads below)
            nc.gpsimd.memset(v_sb[:, :, :, 0:1], 1.0)
            nc.gpsimd.memset(v_sb[:, :, :, PH + 1:64], 0.0)

            def qkT_proj_chunks(p):
                """4 emit-closures, one per (q/k slot, 2-chunk group)."""
                chunks = []
                for sl in range(2):
                    for tc2 in range(T // (2 * QCW)):
                        def emit(sl=sl, tc2=tc2):
                            ps = stpool.tile([128, 2, QCW], F32, tag="st",
                                             name="pjqk")
                            col0 = (p * 2 + sl) * 128
                            for half in range(2):
                                tcI = tc2 * 2 + half
                                for dt_i in range(NDT):
                                    nc.tensor.matmul(
                                        ps[:, half, :],
                                        wqk_sb[:, dt_i, col0:col0 + 128],
                                        xt_sb[:, dt_i, tcI * QCW:(tcI + 1) * QCW],
                                        start=(dt_i == 0), stop=(dt_i == NDT - 1),
                                        skip_group_check=True)
                            t0 = tc2 * 2 * QCW
                            nc.scalar.activation(
                                qk_sb[:, p, sl, t0:t0 + 2 * QCW].rearrange(
                                    "p (a b) -> p a b", a=2),
                                ps[:, :, :], AF.Identity,
                                bias=bqk_sb[:, p * 2 + sl:p * 2 + sl + 1])
                        chunks.append(emit)
                return chunks

            def vproj_chunks():
                """8 emit-closures, one per tt pair."""
                chunks = []
                for tp in range(KT // 2):
                    def emit(tp=tp):
                        ps = stpool.tile([128, 2, QCW], F32, tag="st",
                                         name="pjv")
                        for half in range(2):
                            tt = tp * 2 + half
                            for dt_i in range(NDT):
                                nc.tensor.matmul(
                                    ps[:, half, 0:HC * PH],
                                    xt_sb[:, dt_i, tt * 128:(tt + 1) * 128],
                                    wv_sb[:, dt_i, :],
                                    start=(dt_i == 0), stop=(dt_i == NDT - 1),
                                    skip_group_check=True)
                        for half in range(2):
                            tt = tp * 2 + half
                            nc.scalar.activation(
                                v_sb[:, tt, :, 1:PH + 1],
                                ps[:, half, 0:HC * PH].rearrange(
                                    "p (h d) -> p h d", h=HC),
                                AF.Copy)
                    chunks.append(emit)
                return chunks

            NB = 8  # kt per batch; QK/AV alternate in batches of NB so PE
            #         tiling-mode switches happen a few times per combo,
            #         not per kt

            class Combo:
                """One (pair, q-group) attention unit, emitted in 4-kt
                batches. AV batches lag QK by 2 batch slots so every exp is
                long done when its AV runs (no per-kt stalls for the
                scheduler to fill with mode-thrashing reordering)."""

                def __init__(self, p, qg, dve):
                    self.p, self.qg, self.dve = p, qg, dve
                    self.cs = np.s_[qg * QCW:(qg + 1) * QCW]
                    self.av = avpool.tile([128, QCW], F32, tag="av",
                                          name="av")
                    self.pts = {}
                    self.r2a = npool.tile([1, QCW], F32, tag="r2a")
                    self.r2b = npool.tile([1, QCW], F32, tag="r2b")
                    self.lrb = npool.tile([1, QCW], F32, tag="lrb")
                    self.rbca = npool.tile([64, QCW], F32, tag="rbca")
                    self.rbcb = npool.tile([64, QCW], F32, tag="rbcb")

                def qk_batch(self, j):
                    for kt in range(j * NB, (j + 1) * NB):
                        st = stpool.tile([128, 2, QCW], F32, tag="st",
                                         name="st")
                        for hh in range(2):
                            b = hh * 64
                            nc.tensor.matmul(
                                st[:, hh, :],
                                qk_sb[b:b + 64, self.p, 1,
                                      kt * 128:(kt + 1) * 128],
                                qk_sb[b:b + 64, self.p, 0, self.cs],
                                start=True, stop=True,
                                skip_group_check=True)
                        pt = ptpool.tile([128, 2, QCW], BF16, tag="pt",
                                         name="pt")
                        if kt % 16 in self.dve:
                            nc.vector.tensor_scalar(
                                pt[:].bitcast(I16), st[:], SCH_A, SCH_B,
                                OP.mult, OP.add)
                        else:
                            nc.scalar.activation(pt[:], st[:], AF.Exp)
                        self.pts[kt] = pt

                def av_batch(self, j):
                    for kt in range(j * NB, (j + 1) * NB):
                        pt = self.pts.pop(kt)
                        for hh in range(2):
                            nc.tensor.matmul(
                                self.av[hh * 64:(hh + 1) * 64, :],
                                v_sb[:, kt, self.p * 2 + hh, :],
                                pt[:, hh, :],
                                start=(kt == 0), stop=(kt == KT - 1),
                                skip_group_check=True)

                def norm_recips(self):
                    # denominators live in av rows 0 / 64
                    nc.vector.reciprocal_approx_fast(self.r2a[:],
                                                     self.av[0:1, :])
                    nc.vector.tensor_copy(self.lrb[:], self.av[64:65, :])
                    nc.vector.reciprocal_approx_fast(self.r2b[:],
                                                     self.lrb[:])
                    nc.gpsimd.partition_broadcast(self.rbca[:], self.r2a[:])
                    nc.gpsimd.partition_broadcast(self.rbcb[:], self.r2b[:])

                def norm_muls(self):
                    # pad rows: av 49-63/113-127 are exact zeros (V' pad
                    # cols) so outT pads come out zero without any memset
                    nc.vector.tensor_mul(outT_sb[0:64, self.p, self.cs],
                                         self.av[0:64, :], self.rbca[:, :])
                    nc.vector.tensor_mul(outT_sb[64:128, self.p, self.cs],
                                         self.av[64:128, :], self.rbcb[:, :])

            def final_proj_chunks(qg):
                """4 emit-closures, one per 128-token tile."""
                chunks = []
                for tt in range(QCW // 128):
                    def emit(tt=tt):
                        t0 = qg * QCW + tt * 128
                        ysb = ypool.tile([128, DIM], F32, tag="ysb")
                        ps = stpool.tile([128, 2, QCW], F32, tag="st",
                                         name="pjy")
                        for jc in range(2):
                            for p2 in range(NP):
                                nc.tensor.matmul(
                                    ps[:, jc, 0:384],
                                    outT_sb[:, p2, t0:t0 + 128],
                                    wo_sb[:, p2, jc * 384:(jc + 1) * 384],
                                    start=(p2 == 0), stop=(p2 == NP - 1),
                                    skip_group_check=True)
                        yv = ysb[:].rearrange("p (a b) -> p a b", a=2)
                        if tt % 2 == 0:
                            nc.scalar.copy(yv, ps[:, :, 0:384])
                        else:
                            nc.vector.tensor_copy(yv, ps[:, :, 0:384])
                        nc.sync.dma_start(y_d[t0:t0 + 128, :], ysb[:])
                    chunks.append(emit)
                return chunks

            # ---- emission order (scheduling priority) ----
            # Pipeline: combo i's QK batches interleave with combo i-1's
            # trailing AV batches and its own leading AV batches; normalize
            # and final-proj chunks slot between batches so serial chains
            # never head-block the exp stream in the FIFO engine queues.
            # qkproj(0), vproj and qkproj(1) go in the DMA-bound prologue.
            for fn in qkT_proj_chunks(0):
                fn()
            for fn in vproj_chunks():
                fn()
            for fn in qkT_proj_chunks(1):
                fn()

            combo_keys = [(qg, p) for qg in range(NQG) for p in range(NP)]
            final_chunks = {}
            prev = None
            for qg, p in combo_keys:
                proj = (qkT_proj_chunks(p + 2)
                        if qg == 0 and p + 2 < NP else [None] * 4)
                if p == 0:
                    final_chunks[qg - 1] = final_proj_chunks(qg - 1) \
                        if qg >= 1 else None
                cur = Combo(p, qg, dve_slots_qg0 if qg == 0 else dve_slots)

                # mode-group structure per combo:
                # [QK kt0-7] [prev AV kt8-15, norm] [proj] [QK kt8-15]
                # [AV kt0-7, prev muls, final] [proj]
                cur.qk_batch(0)
                if prev:
                    prev.av_batch(1)
                    prev.norm_recips()
                if proj[0]:
                    proj[0]()
                if proj[1]:
                    proj[1]()
                cur.qk_batch(1)
                cur.av_batch(0)
                if prev:
                    prev.norm_muls()
                if final_chunks.get(qg - 1):
                    final_chunks[qg - 1][p]()
                if proj[2]:
                    proj[2]()
                if proj[3]:
                    proj[3]()
                prev = cur

            # tail: last combo's trailing AV, normalize, last final projs
            prev.av_batch(1)
            prev.norm_recips()
            prev.norm_muls()
            for fn in final_proj_chunks(NQG - 1):
                fn()

    nc.compile()
    return nc


# ---------------- host-side sharding ----------------

def host_prep(x, w_in, b_in, w_out, T=2048):
    """Full inputs -> list of 8 per-core input dicts."""
    scale = 1.0 / math.sqrt(PH)
    wr = np.asarray(w_in).reshape(DIM, 16, 3, PH)
    br = np.asarray(b_in).reshape(16, 3, PH)
    wog = np.asarray(w_out)  # (768, 768), row dv = h*48+d
    in_maps = []
    for c in range(8):
        b, g = divmod(c, 2)
        wqk = np.zeros((DIM, NP * 2 * 128), np.float32)
        bqk = np.zeros((128, NP * 2), np.float32)
        wv = np.zeros((DIM, HC * PH), np.float32)
        wo = np.zeros((NP * 128, DIM), np.float32)
        for p in range(NP):
            for hh, base in ((0, 0), (1, 64)):
                gh = g * 8 + p * 2 + hh
                wqk[:, (p * 2) * 128 + base:(p * 2) * 128 + base + PH] = wr[:, gh, 0] * scale
                wqk[:, (p * 2 + 1) * 128 + base:(p * 2 + 1) * 128 + base + PH] = wr[:, gh, 1]
                bqk[base:base + PH, p * 2] = br[gh, 0] * scale
                bqk[base:base + PH, p * 2 + 1] = br[gh, 1]
                wv[:, (p * 2 + hh) * PH:(p * 2 + hh + 1) * PH] = wr[:, gh, 2]
                wo[p * 128 + base + 1:p * 128 + base + 1 + PH, :] = wog[gh * PH:(gh + 1) * PH, :]
        in_maps.append({
            "xt": np.ascontiguousarray(np.asarray(x)[b].T).astype(ml_dtypes.bfloat16),
            "wqk": wqk.astype(ml_dtypes.bfloat16),
            "wv": wv.astype(ml_dtypes.bfloat16),
            "wo": wo.astype(ml_dtypes.bfloat16),
            "bqk": bqk,
        })
    return in_maps


def host_post(results, b_out, b_in, w_out, B=4, T=2048):
    # the V bias contributes bv @ w_out, a per-column constant: add on host
    bv_all = np.asarray(b_in).reshape(16, 3, PH)[:, 2, :].reshape(DIM)
    const = np.asarray(b_out) + bv_all @ np.asarray(w_out)
    out = np.empty((B, T, DIM), np.float32)
    for b in range(B):
        out[b] = results[2 * b]["y"] + results[2 * b + 1]["y"] + const[None, :]
    return out


# ---------------- self-contained kernel() entry point ----------------

_CACHED = {}


def _get_nc():
    if "nc" not in _CACHED:
        _CACHED["nc"] = build_kernel(T=2048, num_devices=8)
    return _CACHED["nc"]


def kernel(x, w_in, b_in, w_out, b_out):
    """Full-input MHA forward on 8 NeuronCores.

    x: (4, 2048, 768) f32; w_in: (768, 2304); b_in: (2304,);
    w_out: (768, 768); b_out: (768,). Returns (4, 2048, 768) f32.
    """
    from concourse.bass_utils import run_bass_kernel_spmd

    x = np.asarray(x, np.float32)
    w_in = np.asarray(w_in, np.float32)
    b_in = np.asarray(b_in, np.float32)
    w_out = np.asarray(w_out, np.float32)
    b_out = np.asarray(b_out, np.float32)

    nc = _get_nc()
    in_maps = host_prep(x, w_in, b_in, w_out, T=2048)
    res = run_bass_kernel_spmd(nc, in_maps, core_ids=list(range(8)))
    return host_post(res.results, b_out, b_in, w_out, B=4, T=2048)


# revision 34
# speedup vs baseline: 1.0313x; 1.0046x over previous
"""Bass/Tile MHA kernel for trn2 — builder + host shard/unshard helpers.

Per-core work (8 cores): core c handles batch b=c//2, head-group g=c%2
(8 of 16 heads). v2 design:

- Q^T/K^T are PAIR-PACKED on partitions: head A dims at rows 0-47
  (pad 48-63 = 0), head B at rows 64-111 (pad 112-127 = 0). The QK^T
  matmuls then contract K=64 per head using 64x64 PE array tiling:
  4 concurrent matmuls per k-tile (2 heads x 2 k-token halves) cover
  128 k-tokens x 512 q in ~216ns — 2x less PE time than padding K to
  128. AV matmuls also run as 4 concurrent 64x64 tiles (2 heads x 2
  token halves); the token halves accumulate into separate PSUM banks
  (av_lo/av_hi) which VectorE sums during the softmax normalize.
  All attention matmuls share one (64,64) tiling mode => no PE drains.
- Projections run as batched (128,128)-mode matmul groups between
  attention sections (mode switches cost ~300ns, so they are batched,
  never interleaved per k-tile).
- exp processes BOTH heads' scores in ONE instruction over a 2-bank
  PSUM tile [128, 2, 512] (halves instruction overhead); split between
  ScalarE true-exp and VectorE Schraudolph fast-exp per-kt (tunable).
- No big zero-memsets: the projection activation writes all 128
  partitions; pad rows get bias 0 on zero wqk columns => exact zeros.

Dataflow (all matmuls bf16 in / fp32 PSUM accumulate):
  qkT[d_h, t]  = w_qk^T x + b      (pair-packed, full-128 activation)
  V[t, d_v]    = x w_v             (ones col 0 gives softmax denoms)
  S^T[k, q]    = (K^T)^T Q^T       (4x 64x64 concurrent tiles)
  P^T          = exp(S^T)          (ScalarE exp / VectorE fast-exp)
  av_lo/hi     = (V|1)^T P^T       (4x 64x64 tiles, token halves)
  outT         = (av_lo+av_hi) * bcast(1/l)
  y[t, j]      = outT^T w_out      (+ b_out and cross-core sum on host)
"""

import math

import numpy as np
import ml_dtypes

import concourse.bass as bass
import concourse.mybir as mybir
import concourse.tile as tile
from concourse import bacc

F32 = mybir.dt.float32
BF16 = mybir.dt.bfloat16
I16 = mybir.dt.int16
AF = mybir.ActivationFunctionType
OP = mybir.AluOpType

DIM = 768
PH = 48
NP = 4          # head pairs per core
HC = 8          # heads per core
NDT = DIM // 128  # 6 contraction tiles for the projections

# Schraudolph fast-exp in bf16 bit space: bits = round(x*128/ln2 + (127*128 - C))
SCH_A = 128.0 / math.log(2.0)
SCH_C = 4.7
# +0.5: the fp32->int16 convert truncates, this re-centers it to round-nearest
SCH_B = 127.0 * 128.0 - SCH_C + 0.5

# Which kt slots (mod 16) run fast-exp on VectorE instead of ScalarE exp.
# qg==0 attention overlaps the next pair's projections (ScalarE busier
# with activation copies), so it gives VectorE a larger share.
DVE_SLOTS_QG0 = frozenset({1, 3, 5, 7, 9, 11, 13, 15})
DVE_SLOTS = frozenset({2, 5, 7, 10, 13, 15})


def build_kernel(T=2048, num_devices=8,
                 dve_slots_qg0=DVE_SLOTS_QG0, dve_slots=DVE_SLOTS):
    """Returns compiled Bacc module."""
    KT = T // 128                 # k-tiles (token tiles)
    QCW = min(512, T)             # q chunk width (one PSUM bank)
    NQG = T // QCW                # q groups, one chunk each

    nc = bacc.Bacc("TRN2", target_bir_lowering=False, debug=False,
                   num_devices=num_devices)

    xt_d = nc.dram_tensor("xt", (DIM, T), BF16, kind="ExternalInput")
    wqk_d = nc.dram_tensor("wqk", (DIM, NP * 2 * 128), BF16, kind="ExternalInput")
    wv_d = nc.dram_tensor("wv", (DIM, HC * PH), BF16, kind="ExternalInput")
    wo_d = nc.dram_tensor("wo", (NP * 128, DIM), BF16, kind="ExternalInput")
    bqk_d = nc.dram_tensor("bqk", (128, NP * 2), F32, kind="ExternalInput")
    y_d = nc.dram_tensor("y", (T, DIM), F32, kind="ExternalOutput")

    with tile.TileContext(nc) as tc:
        with (
            tc.tile_pool(name="const", bufs=1) as cpool,
            tc.tile_pool(name="pt", bufs=20) as ptpool,
            tc.tile_pool(name="norm", bufs=2) as npool,
            tc.tile_pool(name="ysb", bufs=2) as ypool,
            tc.tile_pool(name="st", bufs=3, space="PSUM") as stpool,
            tc.tile_pool(name="av", bufs=2, space="PSUM") as avpool,
        ):
            # ---- persistent SBUF tensors ----
            xt_sb = cpool.tile([128, NDT, T], BF16, tag="xt")
            wqk_sb = cpool.tile([128, NDT, NP * 2 * 128], BF16, tag="wqk")
            wv_sb = cpool.tile([128, NDT, HC * PH], BF16, tag="wv")
            wo_sb = cpool.tile([128, NP, DIM], BF16, tag="wo")
            bqk_sb = cpool.tile([128, NP * 2], F32, tag="bqk")
            # pair-packed Q^T/K^T: rows 0-47 head A (48-63 zero), 64-111
            # head B (112-127 zero); slot 0 = Q^T, slot 1 = K^T
            qk_sb = cpool.tile([128, NP, 2, T], BF16, tag="qk")
            # V' columns per head: 0 = ones (softmax denominator lands on
            # PSUM row 0 / 64 of av_lo+av_hi), 1-48 = V, 49-63 = zero
            v_sb = cpool.tile([128, KT, HC, 64], BF16, tag="v")
            outT_sb = cpool.tile([128, NP, T], BF16, tag="outT")

            # ---- input DMAs ----
            # xt + wqk + bqk first (the first projection chunk needs them),
            # wv/wo later (only needed once attention starts)
            nc.sync.dma_start(bqk_sb[:], bqk_d[:])
            for dt_i in range(NDT):
                nc.sync.dma_start(xt_sb[:, dt_i, :], xt_d[dt_i * 128:(dt_i + 1) * 128, :])
                nc.sync.dma_start(wqk_sb[:, dt_i, :], wqk_d[dt_i * 128:(dt_i + 1) * 128, :])
            for dt_i in range(NDT):
                nc.sync.dma_start(wv_sb[:, dt_i, :], wv_d[dt_i * 128:(dt_i + 1) * 128, :])
            for p in range(NP):
                nc.sync.dma_start(wo_sb[:, p, :], wo_d[p * 128:(p + 1) * 128, :])

            # ones column for the softmax-denominator trick; zero pads keep
            # av rows 49-63 / 113-127 finite (they multiply rbc pads below)
            nc.gpsimd.memset(v_sb[:, :, :, 0:1], 1.0)
            nc.gpsimd.memset(v_sb[:, :, :, PH + 1:64], 0.0)

            def qkT_proj_chunks(p):
                """4 emit-closures, one per (q/k slot, 2-chunk group)."""
                chunks = []
                for sl in range(2):
                    for tc2 in range(T // (2 * QCW)):
                        def emit(sl=sl, tc2=tc2):
                            ps = stpool.tile([128, 2, QCW], F32, tag="st",
                                             name="pjqk")
                            col0 = (p * 2 + sl) * 128
                            for half in range(2):
                                tcI = tc2 * 2 + half
                                for dt_i in range(NDT):
                                    nc.tensor.matmul(
                                        ps[:, half, :],
                                        wqk_sb[:, dt_i, col0:col0 + 128],
                                        xt_sb[:, dt_i, tcI * QCW:(tcI + 1) * QCW],
                                        start=(dt_i == 0), stop=(dt_i == NDT - 1),
                                        skip_group_check=True)
                            t0 = tc2 * 2 * QCW
                            nc.scalar.activation(
                                qk_sb[:, p, sl, t0:t0 + 2 * QCW].rearrange(
                                    "p (a b) -> p a b", a=2),
                                ps[:, :, :], AF.Identity,
                                bias=bqk_sb[:, p * 2 + sl:p * 2 + sl + 1])
                        chunks.append(emit)
                return chunks

            def vproj_chunks():
                """8 emit-closures, one per tt pair."""
                chunks = []
                for tp in range(KT // 2):
                    def emit(tp=tp):
                        ps = stpool.tile([128, 2, QCW], F32, tag="st",
                                         name="pjv")
                        for half in range(2):
                            tt = tp * 2 + half
                            for dt_i in range(NDT):
                                nc.tensor.matmul(
                                    ps[:, half, 0:HC * PH],
                                    xt_sb[:, dt_i, tt * 128:(tt + 1) * 128],
                                    wv_sb[:, dt_i, :],
                                    start=(dt_i == 0), stop=(dt_i == NDT - 1),
                                    skip_group_check=True)
                        for half in range(2):
                            tt = tp * 2 + half
                            nc.scalar.activation(
                                v_sb[:, tt, :, 1:PH + 1],
                                ps[:, half, 0:HC * PH].rearrange(
                                    "p (h d) -> p h d", h=HC),
                                AF.Copy)
                    chunks.append(emit)
                return chunks

            NB = 8  # kt per batch; QK/AV alternate in batches of NB so PE
            #         tiling-mode switches happen a few times per combo,
            #         not per kt

            class Combo:
                """One (pair, q-group) attention unit, emitted in 4-kt
                batches. AV batches lag QK by 2 batch slots so every exp is
                long done when its AV runs (no per-kt stalls for the
                scheduler to fill with mode-thrashing reordering)."""

                def __init__(self, p, qg, dve):
                    self.p, self.qg, self.dve = p, qg, dve
                    self.cs = np.s_[qg * QCW:(qg + 1) * QCW]
                    self.av = avpool.tile([128, QCW], F32, tag="av",
                                          name="av")
                    self.pts = {}
                    self.r2a = npool.tile([1, QCW], F32, tag="r2a")
                    self.r2b = npool.tile([1, QCW], F32, tag="r2b")
                    self.lrb = npool.tile([1, QCW], F32, tag="lrb")
                    self.rbca = npool.tile([64, QCW], F32, tag="rbca")
                    self.rbcb = npool.tile([64, QCW], F32, tag="rbcb")

                def qk_batch(self, j):
                    for kt in range(j * NB, (j + 1) * NB):
                        st = stpool.tile([128, 2, QCW], F32, tag="st",
                                         name="st")
                        for hh in range(2):
                            b = hh * 64
                            nc.tensor.matmul(
                                st[:, hh, :],
                                qk_sb[b:b + 64, self.p, 1,
                                      kt * 128:(kt + 1) * 128],
                                qk_sb[b:b + 64, self.p, 0, self.cs],
                                start=True, stop=True,
                                skip_group_check=True)
                        pt = ptpool.tile([128, 2, QCW], BF16, tag="pt",
                                         name="pt")
                        if kt % 16 in self.dve:
                            nc.vector.tensor_scalar(
                                pt[:].bitcast(I16), st[:], SCH_A, SCH_B,
                                OP.mult, OP.add)
                        else:
                            nc.scalar.activation(pt[:], st[:], AF.Exp)
                        self.pts[kt] = pt

                def av_batch(self, j):
                    for kt in range(j * NB, (j + 1) * NB):
                        pt = self.pts.pop(kt)
                        for hh in range(2):
                            nc.tensor.matmul(
                                self.av[hh * 64:(hh + 1) * 64, :],
                                v_sb[:, kt, self.p * 2 + hh, :],
                                pt[:, hh, :],
                                start=(kt == 0), stop=(kt == KT - 1),
                                skip_group_check=True)

                def norm_recips(self):
                    # denominators live in av rows 0 / 64
                    nc.vector.reciprocal_approx_fast(self.r2a[:],
                                                     self.av[0:1, :])
                    nc.vector.tensor_copy(self.lrb[:], self.av[64:65, :])
                    nc.vector.reciprocal_approx_fast(self.r2b[:],
                                                     self.lrb[:])
                    nc.gpsimd.partition_broadcast(self.rbca[:], self.r2a[:])
                    nc.gpsimd.partition_broadcast(self.rbcb[:], self.r2b[:])

                def norm_muls(self):
                    # pad rows: av 49-63/113-127 are exact zeros (V' pad
                    # cols) so outT pads come out zero without any memset
                    nc.vector.tensor_mul(outT_sb[0:64, self.p, self.cs],
                                         self.av[0:64, :], self.rbca[:, :])
                    nc.vector.tensor_mul(outT_sb[64:128, self.p, self.cs],
                                         self.av[64:128, :], self.rbcb[:, :])

            def final_proj_chunks(qg):
                """4 emit-closures, one per 128-token tile."""
                chunks = []
                for tt in range(QCW // 128):
                    def emit(tt=tt):
                        t0 = qg * QCW + tt * 128
                        ysb = ypool.tile([128, DIM], F32, tag="ysb")
                        ps = stpool.tile([128, 2, QCW], F32, tag="st",
                                         name="pjy")
                        for jc in range(2):
                            for p2 in range(NP):
                                nc.tensor.matmul(
                                    ps[:, jc, 0:384],
                                    outT_sb[:, p2, t0:t0 + 128],
                                    wo_sb[:, p2, jc * 384:(jc + 1) * 384],
                                    start=(p2 == 0), stop=(p2 == NP - 1),
                                    skip_group_check=True)
                        yv = ysb[:].rearrange("p (a b) -> p a b", a=2)
                        if tt % 2 == 0:
                            nc.scalar.copy(yv, ps[:, :, 0:384])
                        else:
                            nc.vector.tensor_copy(yv, ps[:, :, 0:384])
                        nc.sync.dma_start(y_d[t0:t0 + 128, :], ysb[:])
                    chunks.append(emit)
                return chunks

            # ---- emission order (scheduling priority) ----
            # Pipeline: combo i's QK batches interleave with combo i-1's
            # trailing AV batches and its own leading AV batches; normalize
            # and final-proj chunks slot between batches so serial chains
            # never head-block the exp stream in the FIFO engine queues.
            # qkproj(0), vproj and qkproj(1) go in the DMA-bound prologue.
            for fn in qkT_proj_chunks(0):
                fn()
            for fn in vproj_chunks():
                fn()
            for fn in qkT_proj_chunks(1):
                fn()

            combo_keys = [(qg, p) for qg in range(NQG) for p in range(NP)]
            final_chunks = {}
            prev = None
            for qg, p in combo_keys:
                proj = (qkT_proj_chunks(p + 2)
                        if qg == 0 and p + 2 < NP else [None] * 4)
                if p == 0:
                    final_chunks[qg - 1] = final_proj_chunks(qg - 1) \
                        if qg >= 1 else None
                cur = Combo(p, qg, dve_slots_qg0 if qg == 0 else dve_slots)

                # mode-group structure per combo:
                # [QK kt0-7] [prev AV kt8-15, norm] [proj] [QK kt8-15]
                # [AV kt0-7, prev muls, final] [proj]
                cur.qk_batch(0)
                if prev:
                    prev.av_batch(1)
                    prev.norm_recips()
                if proj[0]:
                    proj[0]()
                if proj[1]:
                    proj[1]()
                cur.qk_batch(1)
                cur.av_batch(0)
                if prev:
                    prev.norm_muls()
                if final_chunks.get(qg - 1):
                    final_chunks[qg - 1][p]()
                if proj[2]:
                    proj[2]()
                if proj[3]:
                    proj[3]()
                prev = cur

            # tail: last combo's trailing AV, normalize, last final projs
            prev.av_batch(1)
            prev.norm_recips()
            prev.norm_muls()
            for fn in final_proj_chunks(NQG - 1):
                fn()

    nc.compile()
    return nc


# ---------------- host-side sharding ----------------

def host_prep(x, w_in, b_in, w_out, T=2048):
    """Full inputs -> list of 8 per-core input dicts."""
    scale = 1.0 / math.sqrt(PH)
    wr = np.asarray(w_in).reshape(DIM, 16, 3, PH)
    br = np.asarray(b_in).reshape(16, 3, PH)
    wog = np.asarray(w_out)  # (768, 768), row dv = h*48+d
    in_maps = []
    for c in range(8):
        b, g = divmod(c, 2)
        wqk = np.zeros((DIM, NP * 2 * 128), np.float32)
        bqk = np.zeros((128, NP * 2), np.float32)
        wv = np.zeros((DIM, HC * PH), np.float32)
        wo = np.zeros((NP * 128, DIM), np.float32)
        for p in range(NP):
            for hh, base in ((0, 0), (1, 64)):
                gh = g * 8 + p * 2 + hh
                wqk[:, (p * 2) * 128 + base:(p * 2) * 128 + base + PH] = wr[:, gh, 0] * scale
                wqk[:, (p * 2 + 1) * 128 + base:(p * 2 + 1) * 128 + base + PH] = wr[:, gh, 1]
                bqk[base:base + PH, p * 2] = br[gh, 0] * scale
                bqk[base:base + PH, p * 2 + 1] = br[gh, 1]
                wv[:, (p * 2 + hh) * PH:(p * 2 + hh + 1) * PH] = wr[:, gh, 2]
                wo[p * 128 + base + 1:p * 128 + base + 1 + PH, :] = wog[gh * PH:(gh + 1) * PH, :]
        in_maps.append({
            "xt": np.ascontiguousarray(np.asarray(x)[b].T).astype(ml_dtypes.bfloat16),
            "wqk": wqk.astype(ml_dtypes.bfloat16),
            "wv": wv.astype(ml_dtypes.bfloat16),
            "wo": wo.astype(ml_dtypes.bfloat16),
            "bqk": bqk,
        })
    return in_maps


def host_post(results, b_out, b_in, w_out, B=4, T=2048):
    # the V bias contributes bv @ w_out, a per-column constant: add on host
    bv_all = np.asarray(b_in).reshape(16, 3, PH)[:, 2, :].reshape(DIM)
    const = np.asarray(b_out) + bv_all @ np.asarray(w_out)
    out = np.empty((B, T, DIM), np.float32)
    for b in range(B):
        out[b] = results[2 * b]["y"] + results[2 * b + 1]["y"] + const[None, :]
    return out


# ---------------- self-contained kernel() entry point ----------------

_CACHED = {}


def _get_nc():
    if "nc" not in _CACHED:
        _CACHED["nc"] = build_kernel(T=2048, num_devices=8)
    return _CACHED["nc"]


def kernel(x, w_in, b_in, w_out, b_out):
    """Full-input MHA forward on 8 NeuronCores.

    x: (4, 2048, 768) f32; w_in: (768, 2304); b_in: (2304,);
    w_out: (768, 768); b_out: (768,). Returns (4, 2048, 768) f32.
    """
    from concourse.bass_utils import run_bass_kernel_spmd

    x = np.asarray(x, np.float32)
    w_in = np.asarray(w_in, np.float32)
    b_in = np.asarray(b_in, np.float32)
    w_out = np.asarray(w_out, np.float32)
    b_out = np.asarray(b_out, np.float32)

    nc = _get_nc()
    in_maps = host_prep(x, w_in, b_in, w_out, T=2048)
    res = run_bass_kernel_spmd(nc, in_maps, core_ids=list(range(8)))
    return host_post(res.results, b_out, b_in, w_out, B=4, T=2048)


# revision 37
# speedup vs baseline: 1.0447x; 1.0130x over previous
"""Bass/Tile MHA kernel for trn2 — builder + host shard/unshard helpers.

Per-core work (8 cores): core c handles batch b=c//2, head-group g=c%2
(8 of 16 heads). v2 design:

- Q^T/K^T are PAIR-PACKED on partitions: head A dims at rows 0-47
  (pad 48-63 = 0), head B at rows 64-111 (pad 112-127 = 0). The QK^T
  matmuls then contract K=64 per head using 64x64 PE array tiling:
  4 concurrent matmuls per k-tile (2 heads x 2 k-token halves) cover
  128 k-tokens x 512 q in ~216ns — 2x less PE time than padding K to
  128. AV matmuls also run as 4 concurrent 64x64 tiles (2 heads x 2
  token halves); the token halves accumulate into separate PSUM banks
  (av_lo/av_hi) which VectorE sums during the softmax normalize.
  All attention matmuls share one (64,64) tiling mode => no PE drains.
- Projections run as batched (128,128)-mode matmul groups between
  attention sections (mode switches cost ~300ns, so they are batched,
  never interleaved per k-tile).
- exp processes BOTH heads' scores in ONE instruction over a 2-bank
  PSUM tile [128, 2, 512] (halves instruction overhead); split between
  ScalarE true-exp and VectorE Schraudolph fast-exp per-kt (tunable).
- No big zero-memsets: the projection activation writes all 128
  partitions; pad rows get bias 0 on zero wqk columns => exact zeros.

Dataflow (all matmuls bf16 in / fp32 PSUM accumulate):
  qkT[d_h, t]  = w_qk^T x + b      (pair-packed, full-128 activation)
  V[t, d_v]    = x w_v             (ones col 0 gives softmax denoms)
  S^T[k, q]    = (K^T)^T Q^T       (4x 64x64 concurrent tiles)
  P^T          = exp(S^T)          (ScalarE exp / VectorE fast-exp)
  av_lo/hi     = (V|1)^T P^T       (4x 64x64 tiles, token halves)
  outT         = (av_lo+av_hi) * bcast(1/l)
  y[t, j]      = outT^T w_out      (+ b_out and cross-core sum on host)
"""

import math

import numpy as np
import ml_dtypes

import concourse.bass as bass
import concourse.mybir as mybir
import concourse.tile as tile
from concourse import bacc

F32 = mybir.dt.float32
BF16 = mybir.dt.bfloat16
I16 = mybir.dt.int16
AF = mybir.ActivationFunctionType
OP = mybir.AluOpType

DIM = 768
PH = 48
NP = 4          # head pairs per core
HC = 8          # heads per core
NDT = DIM // 128  # 6 contraction tiles for the projections

# Schraudolph fast-exp in bf16 bit space: bits = round(x*128/ln2 + (127*128 - C))
SCH_A = 128.0 / math.log(2.0)
SCH_C = 4.7
# +0.5: the fp32->int16 convert truncates, this re-centers it to round-nearest
SCH_B = 127.0 * 128.0 - SCH_C + 0.5

# Which kt slots (mod 16) run fast-exp on VectorE instead of ScalarE exp.
# qg==0 attention overlaps the next pair's projections (ScalarE busier
# with activation copies), so it gives VectorE a larger share.
DVE_SLOTS_QG0 = frozenset({1, 3, 5, 7, 9, 11, 13, 15})
DVE_SLOTS = frozenset({2, 5, 7, 10, 13, 15})


def build_kernel(T=2048, num_devices=8,
                 dve_slots_qg0=DVE_SLOTS_QG0, dve_slots=DVE_SLOTS):
    """Returns compiled Bacc module."""
    KT = T // 128                 # k-tiles (token tiles)
    QCW = min(512, T)             # q chunk width (one PSUM bank)
    NQG = T // QCW                # q groups, one chunk each

    nc = bacc.Bacc("TRN2", target_bir_lowering=False, debug=False,
                   num_devices=num_devices)

    xt_d = nc.dram_tensor("xt", (DIM, T), BF16, kind="ExternalInput")
    wqk_d = nc.dram_tensor("wqk", (DIM, NP * 2 * 128), BF16, kind="ExternalInput")
    wv_d = nc.dram_tensor("wv", (DIM, HC * PH), BF16, kind="ExternalInput")
    wo_d = nc.dram_tensor("wo", (NP * 128, DIM), BF16, kind="ExternalInput")
    bqk_d = nc.dram_tensor("bqk", (128, NP * 2), F32, kind="ExternalInput")
    y_d = nc.dram_tensor("y", (T, DIM), F32, kind="ExternalOutput")

    with tile.TileContext(nc) as tc:
        with (
            tc.tile_pool(name="const", bufs=1) as cpool,
            tc.tile_pool(name="pt", bufs=20) as ptpool,
            tc.tile_pool(name="norm", bufs=2) as npool,
            tc.tile_pool(name="ysb", bufs=4) as ypool,
            tc.tile_pool(name="st", bufs=3, space="PSUM") as stpool,
            tc.tile_pool(name="av", bufs=2, space="PSUM") as avpool,
        ):
            # ---- persistent SBUF tensors ----
            xt_sb = cpool.tile([128, NDT, T], BF16, tag="xt")
            wqk_sb = cpool.tile([128, NDT, NP * 2 * 128], BF16, tag="wqk")
            wv_sb = cpool.tile([128, NDT, HC * PH], BF16, tag="wv")
            wo_sb = cpool.tile([128, NP, DIM], BF16, tag="wo")
            bqk_sb = cpool.tile([128, NP * 2], F32, tag="bqk")
            # pair-packed Q^T/K^T: rows 0-47 head A (48-63 zero), 64-111
            # head B (112-127 zero); slot 0 = Q^T, slot 1 = K^T
            qk_sb = cpool.tile([128, NP, 2, T], BF16, tag="qk")
            # V' columns per head: 0 = ones (softmax denominator lands on
            # PSUM row 0 / 64 of av_lo+av_hi), 1-48 = V, 49-63 = zero
            v_sb = cpool.tile([128, KT, HC, 64], BF16, tag="v")
            outT_sb = cpool.tile([128, NP, T], BF16, tag="outT")

            # ---- input DMAs ----
            # xt + wqk + bqk first (the first projection chunk needs them),
            # wv/wo later (only needed once attention starts)
            nc.sync.dma_start(bqk_sb[:], bqk_d[:])
            for dt_i in range(NDT):
                nc.sync.dma_start(xt_sb[:, dt_i, :], xt_d[dt_i * 128:(dt_i + 1) * 128, :])
                nc.sync.dma_start(wqk_sb[:, dt_i, :], wqk_d[dt_i * 128:(dt_i + 1) * 128, :])
            for dt_i in range(NDT):
                nc.sync.dma_start(wv_sb[:, dt_i, :], wv_d[dt_i * 128:(dt_i + 1) * 128, :])
            for p in range(NP):
                nc.sync.dma_start(wo_sb[:, p, :], wo_d[p * 128:(p + 1) * 128, :])

            # ones column for the softmax-denominator trick; zero pads keep
            # av rows 49-63 / 113-127 finite (they multiply rbc pads below)
            nc.gpsimd.memset(v_sb[:, :, :, 0:1], 1.0)
            nc.gpsimd.memset(v_sb[:, :, :, PH + 1:64], 0.0)

            def qkT_proj_chunks(p):
                """4 emit-closures, one per (q/k slot, 2-chunk group)."""
                chunks = []
                for sl in range(2):
                    for tc2 in range(T // (2 * QCW)):
                        def emit(sl=sl, tc2=tc2):
                            ps = stpool.tile([128, 2, QCW], F32, tag="st",
                                             name="pjqk")
                            col0 = (p * 2 + sl) * 128
                            for half in range(2):
                                tcI = tc2 * 2 + half
                                for dt_i in range(NDT):
                                    nc.tensor.matmul(
                                        ps[:, half, :],
                                        wqk_sb[:, dt_i, col0:col0 + 128],
                                        xt_sb[:, dt_i, tcI * QCW:(tcI + 1) * QCW],
                                        start=(dt_i == 0), stop=(dt_i == NDT - 1),
                                        skip_group_check=True)
                            t0 = tc2 * 2 * QCW
                            nc.scalar.activation(
                                qk_sb[:, p, sl, t0:t0 + 2 * QCW].rearrange(
                                    "p (a b) -> p a b", a=2),
                                ps[:, :, :], AF.Identity,
                                bias=bqk_sb[:, p * 2 + sl:p * 2 + sl + 1])
                        chunks.append(emit)
                return chunks

            def vproj_chunks():
                """8 emit-closures, one per tt pair."""
                chunks = []
                for tp in range(KT // 2):
                    def emit(tp=tp):
                        ps = stpool.tile([128, 2, QCW], F32, tag="st",
                                         name="pjv")
                        for half in range(2):
                            tt = tp * 2 + half
                            for dt_i in range(NDT):
                                nc.tensor.matmul(
                                    ps[:, half, 0:HC * PH],
                                    xt_sb[:, dt_i, tt * 128:(tt + 1) * 128],
                                    wv_sb[:, dt_i, :],
                                    start=(dt_i == 0), stop=(dt_i == NDT - 1),
                                    skip_group_check=True)
                        for half in range(2):
                            tt = tp * 2 + half
                            nc.scalar.activation(
                                v_sb[:, tt, :, 1:PH + 1],
                                ps[:, half, 0:HC * PH].rearrange(
                                    "p (h d) -> p h d", h=HC),
                                AF.Copy)
                    chunks.append(emit)
                return chunks

            NB = 8  # kt per batch; QK/AV alternate in batches of NB so PE
            #         tiling-mode switches happen a few times per combo,
            #         not per kt

            class Combo:
                """One (pair, q-group) attention unit, emitted in 4-kt
                batches. AV batches lag QK by 2 batch slots so every exp is
                long done when its AV runs (no per-kt stalls for the
                scheduler to fill with mode-thrashing reordering)."""

                def __init__(self, p, qg, dve):
                    self.p, self.qg, self.dve = p, qg, dve
                    self.cs = np.s_[qg * QCW:(qg + 1) * QCW]
                    self.av = avpool.tile([128, QCW], F32, tag="av",
                                          name="av")
                    self.pts = {}
                    self.r2a = npool.tile([1, QCW], F32, tag="r2a")
                    self.r2b = npool.tile([1, QCW], F32, tag="r2b")
                    self.lrb = npool.tile([1, QCW], F32, tag="lrb")
                    self.rbca = npool.tile([64, QCW], F32, tag="rbca")
                    self.rbcb = npool.tile([64, QCW], F32, tag="rbcb")

                def qk_batch(self, j):
                    for kt in range(j * NB, (j + 1) * NB):
                        st = stpool.tile([128, 2, QCW], F32, tag="st",
                                         name="st")
                        for hh in range(2):
                            b = hh * 64
                            nc.tensor.matmul(
                                st[:, hh, :],
                                qk_sb[b:b + 64, self.p, 1,
                                      kt * 128:(kt + 1) * 128],
                                qk_sb[b:b + 64, self.p, 0, self.cs],
                                start=True, stop=True,
                                skip_group_check=True)
                        pt = ptpool.tile([128, 2, QCW], BF16, tag="pt",
                                         name="pt")
                        if kt % 16 in self.dve:
                            nc.vector.tensor_scalar(
                                pt[:].bitcast(I16), st[:], SCH_A, SCH_B,
                                OP.mult, OP.add)
                        else:
                            nc.scalar.activation(pt[:], st[:], AF.Exp)
                        self.pts[kt] = pt

                def av_batch(self, j):
                    for kt in range(j * NB, (j + 1) * NB):
                        pt = self.pts.pop(kt)
                        for hh in range(2):
                            nc.tensor.matmul(
                                self.av[hh * 64:(hh + 1) * 64, :],
                                v_sb[:, kt, self.p * 2 + hh, :],
                                pt[:, hh, :],
                                start=(kt == 0), stop=(kt == KT - 1),
                                skip_group_check=True)

                def norm_recips(self):
                    # denominators live in av rows 0 / 64
                    nc.vector.reciprocal_approx_fast(self.r2a[:],
                                                     self.av[0:1, :])
                    nc.vector.tensor_copy(self.lrb[:], self.av[64:65, :])
                    nc.vector.reciprocal_approx_fast(self.r2b[:],
                                                     self.lrb[:])
                    nc.gpsimd.partition_broadcast(self.rbca[:], self.r2a[:])
                    nc.gpsimd.partition_broadcast(self.rbcb[:], self.r2b[:])

                def norm_muls(self):
                    # pad rows: av 49-63/113-127 are exact zeros (V' pad
                    # cols) so outT pads come out zero without any memset
                    nc.vector.tensor_mul(outT_sb[0:64, self.p, self.cs],
                                         self.av[0:64, :], self.rbca[:, :])
                    nc.vector.tensor_mul(outT_sb[64:128, self.p, self.cs],
                                         self.av[64:128, :], self.rbcb[:, :])

            def final_proj_chunks(qg):
                """4 emit-closures, one per 128-token tile."""
                chunks = []
                for tt in range(QCW // 128):
                    def emit(tt=tt):
                        t0 = qg * QCW + tt * 128
                        ysb = ypool.tile([128, DIM], F32, tag="ysb")
                        ps = stpool.tile([128, 2, QCW], F32, tag="st",
                                         name="pjy")
                        for jc in range(2):
                            for p2 in range(NP):
                                nc.tensor.matmul(
                                    ps[:, jc, 0:384],
                                    outT_sb[:, p2, t0:t0 + 128],
                                    wo_sb[:, p2, jc * 384:(jc + 1) * 384],
                                    start=(p2 == 0), stop=(p2 == NP - 1),
                                    skip_group_check=True)
                        yv = ysb[:].rearrange("p (a b) -> p a b", a=2)
                        if tt % 2 == 0:
                            nc.scalar.copy(yv, ps[:, :, 0:384])
                        else:
                            nc.vector.tensor_copy(yv, ps[:, :, 0:384])
                        nc.sync.dma_start(y_d[t0:t0 + 128, :], ysb[:])
                    chunks.append(emit)
                return chunks

            # ---- emission order (scheduling priority) ----
            # Pipeline: combo i's QK batches interleave with combo i-1's
            # trailing AV batches and its own leading AV batches; normalize
            # and final-proj chunks slot between batches so serial chains
            # never head-block the exp stream in the FIFO engine queues.
            # qkproj(0), vproj and qkproj(1) go in the DMA-bound prologue.
            for fn in qkT_proj_chunks(0):
                fn()
            for fn in vproj_chunks():
                fn()
            for fn in qkT_proj_chunks(1):
                fn()

            combo_keys = [(qg, p) for qg in range(NQG) for p in range(NP)]
            final_chunks = {}
            prev = None
            for qg, p in combo_keys:
                proj = (qkT_proj_chunks(p + 2)
                        if qg == 0 and p + 2 < NP else [None] * 4)
                if p == 0:
                    final_chunks[qg - 1] = final_proj_chunks(qg - 1) \
                        if qg >= 1 else None
                if qg == 0:
                    dv = dve_slots_qg0
                elif (qg, p) == (NQG - 1, NP - 1):
                    # last combo: drain both exp engines together
                    dv = frozenset({1, 3, 5, 7, 9, 11, 13})
                else:
                    dv = dve_slots
                cur = Combo(p, qg, dv)

                # mode-group structure per combo:
                # [QK kt0-7] [prev AV kt8-15, norm] [proj] [QK kt8-15]
                # [AV kt0-7, prev muls, final] [proj]
                cur.qk_batch(0)
                if prev:
                    prev.av_batch(1)
                    prev.norm_recips()
                if proj[0]:
                    proj[0]()
                if proj[1]:
                    proj[1]()
                cur.qk_batch(1)
                cur.av_batch(0)
                if prev:
                    prev.norm_muls()
                if final_chunks.get(qg - 1):
                    final_chunks[qg - 1][p]()
                if proj[2]:
                    proj[2]()
                if proj[3]:
                    proj[3]()
                prev = cur

            # tail: last combo's trailing AV, normalize, last final projs.
            # The final matmul groups for 3 of 4 token tiles run while the
            # normalize chain completes (only their last accumulation step
            # waits on outT pair 3); copies and DMAs pipeline behind them.
            prev.av_batch(1)
            prev.norm_recips()
            prev.norm_muls()
            boxes = []

            def tail_mms(tt):
                t0 = (NQG - 1) * QCW + tt * 128
                ysb = ypool.tile([128, DIM], F32, tag="ysb", name="ysbt")
                ps = stpool.tile([128, 2, QCW], F32, tag="st", name="pjyt")
                for jc in range(2):
                    for p2 in range(NP):
                        nc.tensor.matmul(
                            ps[:, jc, 0:384],
                            outT_sb[:, p2, t0:t0 + 128],
                            wo_sb[:, p2, jc * 384:(jc + 1) * 384],
                            start=(p2 == 0), stop=(p2 == NP - 1),
                            skip_group_check=True)
                boxes.append((tt, ps, ysb))

            def tail_copy(i):
                tt, ps, ysb = boxes[i]
                t0 = (NQG - 1) * QCW + tt * 128
                yv = ysb[:].rearrange("p (a b) -> p a b", a=2)
                if tt % 2 == 0:
                    nc.scalar.copy(yv, ps[:, :, 0:384])
                else:
                    nc.vector.tensor_copy(yv, ps[:, :, 0:384])
                nc.sync.dma_start(y_d[t0:t0 + 128, :], ysb[:])

            tail_mms(0)
            tail_mms(1)
            tail_mms(2)
            tail_copy(0)
            tail_mms(3)
            tail_copy(1)
            tail_copy(2)
            tail_copy(3)

    nc.compile()
    return nc


# ---------------- host-side sharding ----------------

def host_prep(x, w_in, b_in, w_out, T=2048):
    """Full inputs -> list of 8 per-core input dicts."""
    scale = 1.0 / math.sqrt(PH)
    wr = np.asarray(w_in).reshape(DIM, 16, 3, PH)
    br = np.asarray(b_in).reshape(16, 3, PH)
    wog = np.asarray(w_out)  # (768, 768), row dv = h*48+d
    in_maps = []
    for c in range(8):
        b, g = divmod(c, 2)
        wqk = np.zeros((DIM, NP * 2 * 128), np.float32)
        bqk = np.zeros((128, NP * 2), np.float32)
        wv = np.zeros((DIM, HC * PH), np.float32)
        wo = np.zeros((NP * 128, DIM), np.float32)
        for p in range(NP):
            for hh, base in ((0, 0), (1, 64)):
                gh = g * 8 + p * 2 + hh
                wqk[:, (p * 2) * 128 + base:(p * 2) * 128 + base + PH] = wr[:, gh, 0] * scale
                wqk[:, (p * 2 + 1) * 128 + base:(p * 2 + 1) * 128 + base + PH] = wr[:, gh, 1]
                bqk[base:base + PH, p * 2] = br[gh, 0] * scale
                bqk[base:base + PH, p * 2 + 1] = br[gh, 1]
                wv[:, (p * 2 + hh) * PH:(p * 2 + hh + 1) * PH] = wr[:, gh, 2]
                wo[p * 128 + base + 1:p * 128 + base + 1 + PH, :] = wog[gh * PH:(gh + 1) * PH, :]
        in_maps.append({
            "xt": np.ascontiguousarray(np.asarray(x)[b].T).astype(ml_dtypes.bfloat16),
            "wqk": wqk.astype(ml_dtypes.bfloat16),
            "wv": wv.astype(ml_dtypes.bfloat16),
            "wo": wo.astype(ml_dtypes.bfloat16),
            "bqk": bqk,
        })
    return in_maps


def host_post(results, b_out, b_in, w_out, B=4, T=2048):
    # the V bias contributes bv @ w_out, a per-column constant: add on host
    bv_all = np.asarray(b_in).reshape(16, 3, PH)[:, 2, :].reshape(DIM)
    const = np.asarray(b_out) + bv_all @ np.asarray(w_out)
    out = np.empty((B, T, DIM), np.float32)
    for b in range(B):
        out[b] = results[2 * b]["y"] + results[2 * b + 1]["y"] + const[None, :]
    return out


# ---------------- self-contained kernel() entry point ----------------

_CACHED = {}


def _get_nc():
    if "nc" not in _CACHED:
        _CACHED["nc"] = build_kernel(T=2048, num_devices=8)
    return _CACHED["nc"]


def kernel(x, w_in, b_in, w_out, b_out):
    """Full-input MHA forward on 8 NeuronCores.

    x: (4, 2048, 768) f32; w_in: (768, 2304); b_in: (2304,);
    w_out: (768, 768); b_out: (768,). Returns (4, 2048, 768) f32.
    """
    from concourse.bass_utils import run_bass_kernel_spmd

    x = np.asarray(x, np.float32)
    w_in = np.asarray(w_in, np.float32)
    b_in = np.asarray(b_in, np.float32)
    w_out = np.asarray(w_out, np.float32)
    b_out = np.asarray(b_out, np.float32)

    nc = _get_nc()
    in_maps = host_prep(x, w_in, b_in, w_out, T=2048)
    res = run_bass_kernel_spmd(nc, in_maps, core_ids=list(range(8)))
    return host_post(res.results, b_out, b_in, w_out, B=4, T=2048)
